# revision 25
# baseline (speedup 1.0000x reference)
"""GCN layer with virtual node on 8 Trainium2 NeuronCores (Bass/Tile).

Reference computation (fp32):
    agg = segment_sum(H[src], dst, N)        # message passing
    out = H + agg
    vmean = segment_mean(out, batch, G)      # virtual node
    out = out + vmean[batch]
    y = relu(out @ W)

Distribution strategy (self-contained, hardcoded):
  - batch is sorted, G=256 graphs, 8 cores -> core c owns graphs
    [32c, 32c+32) == a contiguous node range (graph-aligned node sharding).
    Per-graph means never cross cores: no collectives needed.
  - Edges partitioned by owning core of dst (host-side index arithmetic).
    Within a core: 128-dst windows. Source rows are fetched from a DRAM fp16
    copy of H with gpsimd dma_gather (int16 indices -> 4 source "classes" of
    <=32768 rows). Gather calls are grouped over GW windows per class and
    spread round-robin over 4 SWDGE queues so descriptor generation runs on
    all four Q7 core pairs concurrently (measured ~3.2x vs single queue).
  - segment_sum via PE one-hot matmul in TRANSPOSED orientation:
    psum_wT[f, dst] += G_t^T @ R_t, where R_t[s, m] = (drel[s,t] == m) is
    built on DVE (is_equal vs iota, batched over a whole gather call), so
    out_T[f, node] accumulates in SBUF and the final y = out^T... matmul
    needs no PE transpose.
  - virtual node: per window w, transpose out_T_w back (PE+ident) and
    accumulate psum_sT[f, g] += out_w^T @ b_w with host-baked one-hot
    b_w[node, graph]; vmeanT = psum_sT * (1/count); VW = vmeanT^T @ W.
  - final: psum_y = out_T_w^T @ W + Bt_w^T @ VW (host-baked Bt[g, node]),
    relu on ACT, DMA out.
"""
import os
import numpy as np

from concourse import bacc, mybir
import concourse.tile as tile
from concourse.bass_utils import run_bass_kernel_spmd

P = 128
N_CORES = 8
D = 128
F32 = mybir.dt.float32
I16 = mybir.dt.int16
F16 = mybir.dt.float16
CLASS_SIZE = 32768  # int16 index reach for dma_gather
GW = 3              # windows per gather group (keeps calls under the
                    # 16KB/lane SWDGE ring: ~2300 idxs -> ~145 descs/lane)
NQ = 4              # SWDGE queues


def _ceil(a, b):
    return -(-a // b)


# ---------------------------------------------------------------------------
# host-side prep: pure index arithmetic / sharding metadata
# ---------------------------------------------------------------------------

def _prep(H, edge_index, batch, n_graphs):
    N = H.shape[0]
    src = np.asarray(edge_index[0], dtype=np.int64)
    dst = np.asarray(edge_index[1], dtype=np.int64)
    batch = np.asarray(batch, dtype=np.int64)
    gpc = n_graphs // N_CORES
    n_cls = _ceil(N, CLASS_SIZE)

    gstart = np.searchsorted(batch, np.arange(n_graphs + 1))
    core_start = gstart[::gpc]  # [N_CORES+1]
    counts = np.diff(gstart)

    node_core = (batch // gpc).astype(np.int64)
    ecore = node_core[dst]

    n_c = np.diff(core_start)
    NW = int(_ceil(n_c.max(), P))

    # per-core (w, k) counts and sorted edge runs
    cnt = np.zeros((N_CORES, NW, n_cls), dtype=np.int64)
    runs = []
    for c in range(N_CORES):
        m = ecore == c
        s_c, d_c = src[m], dst[m]
        dstl = d_c - core_start[c]
        w = dstl >> 7
        k = s_c // CLASS_SIZE
        key = w * n_cls + k
        # src-sorted within each (w, k) run: ascending gather addresses
        # give the SDMA engines better HBM row/bank locality.
        order = np.lexsort((s_c, key))
        s_c, dstl, key = s_c[order], dstl[order], key[order]
        cnt[c] = np.bincount(key, minlength=NW * n_cls).reshape(NW, n_cls)
        runs.append((s_c, dstl, key))

    tiles_wk = _ceil(cnt.max(axis=0), P)            # [NW, n_cls]
    tiles_wk[:, 0] = np.maximum(tiles_wk[:, 0], 1)  # every window has >=1 tile

    groups = [(lo, min(lo + GW, NW)) for lo in range(0, NW, GW)]
    # layout: for g: for k (rotated per group): for w in group -> tiles.
    # The class order rotates so the round-robin SWDGE queue assignment
    # (queue = issue order % 4, which must stay in lockstep with Tile's
    # DMASW sem-lane rotation) sees balanced work per queue despite the
    # small last class.
    calls = []  # (gi, k, call_t0, nt, [(w, wt0, ntw), ...])
    wt0 = np.zeros((NW, n_cls), dtype=np.int64)
    t = 0
    for gi, (lo, hi) in enumerate(groups):
        for j in range(n_cls):
            k = (j + gi) % n_cls
            c_t0 = t
            wl = []
            for w in range(lo, hi):
                wt0[w, k] = t
                wl.append((w, t, int(tiles_wk[w, k])))
                t += int(tiles_wk[w, k])
            calls.append((gi, k, c_t0, t - c_t0, wl))
    T = int(t)
    RT = max(nt for _, _, _, nt, _ in calls)

    params = dict(
        N=N, NW=NW, T=T, RT=RT, gpc=gpc, n_cls=n_cls,
        tiles=tuple(tuple(int(x) for x in row) for row in tiles_wk),
        groups=tuple(groups),
        cls_size=tuple(min(CLASS_SIZE, N - CLASS_SIZE * k)
                       for k in range(n_cls)),
    )

    in_maps = []
    h16 = np.ascontiguousarray(H, dtype=np.float16)
    for c in range(N_CORES):
        s_c, dstl, key = runs[c]
        idx_flat = np.full(T * P, -1, dtype=np.int64)
        drel = np.full(T * P, -1.0, dtype=np.float32)
        # slot of each edge: base slot of its (w,k) run + offset within run
        cnt_c = cnt[c].ravel()
        run_start = np.concatenate([[0], np.cumsum(cnt_c)])
        off_in_run = np.arange(len(key)) - run_start[key]
        slot = wt0.ravel()[key] * P + off_in_run
        idx_flat[slot] = s_c - (key % n_cls) * CLASS_SIZE
        drel[slot] = (dstl & 127).astype(np.float32)
        # interior pads (before the last real edge of each call) -> idx 0.
        # num_idxs_reg must equal the per-call count of non-negative idxs
        # (the SWDGE ring reserves space from the register while Q7
        # generates from the trailing-trimmed idx list -- a mismatch
        # corrupts the ring bookkeeping and wedges the device).
        cnt32 = np.zeros(len(calls), dtype=np.int32)
        for ci, (_, _, c_t0, nt, _) in enumerate(calls):
            blk = idx_flat[c_t0 * P:(c_t0 + nt) * P]
            real = np.nonzero(blk >= 0)[0]
            if len(real):
                last = int(real[-1])
            else:
                blk[0] = 0  # keep every call non-empty
                last = 0
            pad = blk[:last + 1] < 0
            blk[:last + 1][pad] = 0
            cnt32[ci] = last + 1
        # wrap idx per call region into [16, nt*8] blocks
        wrapped = np.full((16, T * P // 16), -1, dtype=np.int16)
        for _, _, c_t0, nt, _ in calls:
            blk = idx_flat[c_t0 * P:(c_t0 + nt) * P]
            wrapped[:, c_t0 * 8:(c_t0 + nt) * 8] = \
                blk.reshape(nt * 8, 16).T.astype(np.int16)
        wrapped128 = np.ascontiguousarray(np.tile(wrapped, (8, 1)))

        drel16 = np.ascontiguousarray(
            drel.reshape(T, P).T.astype(np.float16))

        nodes = int(n_c[c])
        hcT = np.zeros((P, NW * P), dtype=np.float32)
        hcT[:, :nodes] = np.asarray(H[core_start[c]:core_start[c] + nodes],
                                    dtype=np.float32).T
        br = np.full(NW * P, -1, dtype=np.int64)
        br[:nodes] = batch[core_start[c]:core_start[c] + nodes] - c * gpc
        ball = (br.reshape(NW, P).T[:, :, None]
                == np.arange(gpc)[None, None, :]).astype(np.float16)
        bt = (np.arange(gpc)[:, None] == br[None, :]).astype(np.float16)
        invc = np.broadcast_to(
            (1.0 / np.maximum(counts[c * gpc:(c + 1) * gpc], 1)
             ).astype(np.float32)[None, :], (P, gpc)).copy()

        in_maps.append({
            "h16": h16,
            "cnt32": np.ascontiguousarray(cnt32.reshape(1, -1)),
            "idx16": wrapped128,
            "drel": drel16,
            "hct": np.ascontiguousarray(hcT),
            "ball": np.ascontiguousarray(ball.reshape(P, NW * gpc)),
            "bt": np.ascontiguousarray(bt),
            "invc": invc,
        })
    return params, in_maps, n_c, core_start


def _consts(params, W):
    RT = params["RT"]
    iota_rep = np.broadcast_to(np.arange(P, dtype=np.float16),
                               (P, RT, P)).reshape(P, RT * P).copy()
    ident = np.eye(P, dtype=np.float16)
    return {"iota_rep": iota_rep, "ident": ident,
            "w16": np.ascontiguousarray(W, dtype=np.float16)}


# ---------------------------------------------------------------------------
# device kernel builder (SPMD: one program, per-core data)
# ---------------------------------------------------------------------------

def _build(params):
    NW, T, RT = params["NW"], params["T"], params["RT"]
    gpc, n_cls = params["gpc"], params["n_cls"]
    tiles = params["tiles"]
    groups = params["groups"]
    cls_size = params["cls_size"]
    N = params["N"]

    # reconstruct the call layout (same order as _prep)
    calls = []
    wt0 = {}
    t = 0
    for gi, (lo, hi) in enumerate(groups):
        for j in range(n_cls):
            k = (j + gi) % n_cls
            c_t0 = t
            wl = []
            for w in range(lo, hi):
                wt0[(w, k)] = t
                wl.append((w, t, tiles[w][k]))
                t += tiles[w][k]
            calls.append((gi, k, c_t0, t - c_t0, wl))
    assert t == T

    # max tiles per group (for idx/drel staging buffers)
    gt_span = []
    for gi in range(len(groups)):
        g_t0 = calls[gi * n_cls][2]
        last = calls[gi * n_cls + n_cls - 1]
        gt_span.append((g_t0, last[2] + last[3]))
    GT = max(b - a for a, b in gt_span)
    RTk = [max(nt for _, k2, _, nt, _ in calls if k2 == k)
           for k in range(n_cls)]

    nc = bacc.Bacc("TRN2", target_bir_lowering=False, debug=False,
                   num_devices=N_CORES, num_swdge_queues=NQ)
    h16_d = nc.dram_tensor("h16", [N, D], F16, kind="ExternalInput")
    cnt_d = nc.dram_tensor("cnt32", [1, len(calls)], mybir.dt.int32,
                           kind="ExternalInput")
    idx_d = nc.dram_tensor("idx16", [P, T * P // 16], I16,
                           kind="ExternalInput")
    drel_d = nc.dram_tensor("drel", [P, T], F16, kind="ExternalInput")
    hct_d = nc.dram_tensor("hct", [P, NW * P], F32, kind="ExternalInput")
    ball_d = nc.dram_tensor("ball", [P, NW * gpc], F16, kind="ExternalInput")
    bt_d = nc.dram_tensor("bt", [gpc, NW * P], F16, kind="ExternalInput")
    invc_d = nc.dram_tensor("invc", [P, gpc], F32, kind="ExternalInput")
    iota_d = nc.dram_tensor("iota_rep", [P, RT * P], F16,
                            kind="ExternalInput")
    ident_d = nc.dram_tensor("ident", [P, P], F16, kind="ExternalInput")
    w_d = nc.dram_tensor("w16", [P, D], F16, kind="ExternalInput")
    y_d = nc.dram_tensor("y", [NW * P, D], F32, kind="ExternalOutput")

    with tile.TileContext(nc) as tc:
        with tc.tile_pool(name="const", bufs=1) as cpool:
            iota_t = cpool.tile([P, RT, P], F16)
            nc.sync.dma_start(out=iota_t[:], in_=iota_d[:])
            cnt_t = cpool.tile([1, len(calls)], mybir.dt.int32)
            nc.sync.dma_start(out=cnt_t[:], in_=cnt_d[:])
            ident_t = cpool.tile([P, P], F16)
            nc.sync.dma_start(out=ident_t[:], in_=ident_d[:])
            w_t = cpool.tile([P, D], F16)
            nc.sync.dma_start(out=w_t[:], in_=w_d[:])
            invc_t = cpool.tile([P, gpc], F32)
            nc.sync.dma_start(out=invc_t[:], in_=invc_d[:])

            out_T = cpool.tile([P, NW, P], F16)
            ow16 = cpool.tile([P, NW, P], F16)
            vmeanT = cpool.tile([P, gpc], F16)
            vw16 = cpool.tile([gpc, D], F16)

            from contextlib import ExitStack
            with ExitStack() as stack:
                ep = stack.enter_context
                idxp = ep(tc.tile_pool(name="idxp", bufs=4))
                drelp = ep(tc.tile_pool(name="drelp", bufs=4))
                gpools = [ep(tc.tile_pool(name=f"g{k}", bufs=2))
                          for k in range(n_cls)]
                rpools = [ep(tc.tile_pool(name=f"r{k}", bufs=2))
                          for k in range(n_cls)]
                hcp = ep(tc.tile_pool(name="hcp", bufs=3))
                bp = ep(tc.tile_pool(name="bp", bufs=3))
                onp = ep(tc.tile_pool(name="onp", bufs=3))
                pwp = ep(tc.tile_pool(name="pw", bufs=3, space="PSUM"))
                powp = ep(tc.tile_pool(name="pow", bufs=2, space="PSUM"))
                ptrp = ep(tc.tile_pool(name="ptr", bufs=2, space="PSUM"))
                psp = ep(tc.tile_pool(name="ps", bufs=1, space="PSUM"))
                gcnt = ep(nc.gpsimd.register("gcnt"))
                psum_sT = psp.tile([P, gpc], F32, space="PSUM")
                n_gather = 0

                for gi, (lo, hi) in enumerate(groups):
                    g_t0, g_t1 = gt_span[gi]
                    gcols = g_t1 - g_t0
                    idx_t = idxp.tile([P, GT * 8], I16, tag="idx")
                    nc.sync.dma_start(
                        out=idx_t[:, :gcols * 8],
                        in_=idx_d[:, g_t0 * 8:g_t1 * 8])
                    drel_t = drelp.tile([P, GT], F16, tag="drel")
                    nc.sync.dma_start(
                        out=drel_t[:, :gcols],
                        in_=drel_d[:, g_t0:g_t1])

                    gts = {}
                    rts = {}
                    for j in range(n_cls):
                        _, k, c_t0, nt, _ = calls[gi * n_cls + j]
                        if nt == 0:
                            continue
                        g16 = gpools[k].tile([P, RTk[k], D], F16,
                                             tag=f"G{k}")
                        if gi < 2 or os.environ.get('GCN_SIM_MEMSET'):
                            # pad slots must be finite: 0 * NaN would poison
                            # the one-hot matmul. After the first rotation of
                            # the 2 buffers, stale content is old gathered
                            # rows (finite).
                            nc.vector.memset(g16[:], 0.0)
                        base = CLASS_SIZE * k
                        ci = gi * n_cls + j
                        nc.gpsimd.load(gcnt, cnt_t[0:1, ci:ci + 1])
                        nc.gpsimd.dma_gather(
                            out_ap=g16[:, :nt, :],
                            in_ap=h16_d[base:base + cls_size[k], :],
                            idxs_ap=idx_t[:, (c_t0 - g_t0) * 8:
                                          (c_t0 - g_t0 + nt) * 8],
                            num_idxs=nt * P,
                            num_idxs_reg=gcnt,
                            elem_size=D,
                            single_packet=False,
                            # queue stays in lockstep with Tile's DMASW
                            # sem-lane rotation (lane = issue order % 8,
                            # sems are locked to one SWDGE queue each)
                            queue_num=n_gather % NQ,
                        )
                        n_gather += 1
                        r16 = rpools[k].tile([P, RTk[k], P], F16,
                                             tag=f"R{k}")
                        nc.vector.tensor_tensor(
                            out=r16[:, :nt, :],
                            in0=drel_t[:, c_t0 - g_t0:c_t0 - g_t0 + nt
                                       ].to_broadcast([P, nt, P]),
                            in1=iota_t[:, :nt, :],
                            op=mybir.AluOpType.is_equal)
                        gts[k] = (g16, c_t0)
                        rts[k] = r16

                    for w in range(lo, hi):
                        psum_w = pwp.tile([P, P], F32, space="PSUM",
                                          tag="pw")
                        seq = []
                        for k in range(n_cls):
                            _, c_t0 = gts[k][0], gts[k][1]
                            for j in range(tiles[w][k]):
                                seq.append((k, wt0[(w, k)] - c_t0 + j))
                        for si, (k, j) in enumerate(seq):
                            nc.tensor.matmul(
                                psum_w[:], gts[k][0][:, j, :],
                                rts[k][:, j, :],
                                start=(si == 0), stop=(si == len(seq) - 1))
                        hc_t = hcp.tile([P, P], F32, tag="hc")
                        nc.sync.dma_start(out=hc_t[:],
                                          in_=hct_d[:, w * P:(w + 1) * P])
                        nc.vector.tensor_tensor(
                            out=out_T[:, w, :], in0=psum_w[:], in1=hc_t[:],
                            op=mybir.AluOpType.add)
                        # transpose back for the virtual-node segment sum
                        ptr_t = ptrp.tile([P, P], F16, space="PSUM",
                                          tag="ptr")
                        nc.tensor.transpose(ptr_t[:], out_T[:, w, :],
                                            ident_t[:])
                        on16 = onp.tile([P, P], F16, tag="on")
                        nc.vector.tensor_copy(on16[:], ptr_t[:])
                        b_t = bp.tile([P, gpc], F16, tag="b")
                        nc.sync.dma_start(
                            out=b_t[:],
                            in_=ball_d[:, w * gpc:(w + 1) * gpc])
                        nc.tensor.matmul(
                            psum_sT[:], on16[:], b_t[:],
                            start=(w == 0), stop=(w == NW - 1),
                            skip_group_check=True)
                        # fold the heavy final matmul into phase 1 (PE is
                        # underutilized while gathers dominate): ow = out @ W
                        psum_ow = powp.tile([P, D], F32, space="PSUM",
                                            tag="pow")
                        nc.tensor.matmul(psum_ow[:], out_T[:, w, :], w_t[:],
                                         start=True, stop=True)
                        nc.vector.tensor_copy(ow16[:, w, :], psum_ow[:])

                nc.vector.tensor_tensor(
                    out=vmeanT[:], in0=psum_sT[:], in1=invc_t[:],
                    op=mybir.AluOpType.mult)

            with tc.tile_pool(name="p3", bufs=6) as p3, \
                 tc.tile_pool(name="pvw", bufs=1, space="PSUM") as pvwp, \
                 tc.tile_pool(name="py", bufs=4, space="PSUM") as pyp:
                psum_vw = pvwp.tile([gpc, D], F32, space="PSUM")
                nc.tensor.matmul(psum_vw[:], vmeanT[:], w_t[:],
                                 start=True, stop=True)
                nc.vector.tensor_copy(vw16[:], psum_vw[:])

                for w in range(NW):
                    bt_t = p3.tile([gpc, P], F16, tag="bt")
                    nc.sync.dma_start(out=bt_t[:],
                                      in_=bt_d[:, w * P:(w + 1) * P])
                    psum_y = pyp.tile([P, D], F32, space="PSUM", tag="py")
                    nc.tensor.matmul(psum_y[:], bt_t[:], vw16[:],
                                     start=True, stop=True)
                    ys_t = p3.tile([P, D], F32, tag="YS")
                    nc.vector.tensor_tensor(
                        out=ys_t[:], in0=psum_y[:], in1=ow16[:, w, :],
                        op=mybir.AluOpType.add)
                    y_t = p3.tile([P, D], F32, tag="Y")
                    nc.scalar.activation(y_t[:], ys_t[:],
                                         mybir.ActivationFunctionType.Relu)
                    nc.sync.dma_start(out=y_d[w * P:(w + 1) * P, :],
                                      in_=y_t[:])
    _finish_compile(nc)
    return nc


def _finish_compile(nc):
    nc.compile()
    # compile()'s tail passes (library-load insertion for the custom DMA
    # instructions) can reintroduce >1 sync wait per instruction, which the
    # TRN2 ISA rejects. Re-split and re-codegen.
    import bass_rust
    bass_rust.generate_event_semaphores(nc)
    nc.codegen_inst_isa_subclasses()


_BUILD_CACHE = {}


def _build_cached(params):
    key = tuple(sorted((k, str(v)) for k, v in params.items()))
    if key not in _BUILD_CACHE:
        _BUILD_CACHE[key] = _build(params)
    return _BUILD_CACHE[key]


def _run(H, edge_index, batch, W, n_graphs, trace=False):
    H = np.asarray(H)
    params, in_maps, n_c, core_start = _prep(H, edge_index, batch, n_graphs)
    consts = _consts(params, np.asarray(W))
    for m in in_maps:
        m.update(consts)
    nc = _build_cached(params)
    res = run_bass_kernel_spmd(nc, in_maps, list(range(N_CORES)), trace=trace)
    N = H.shape[0]
    y = np.empty((N, D), dtype=np.float32)
    for c in range(N_CORES):
        y[core_start[c]:core_start[c] + n_c[c]] = \
            res.results[c]["y"][:n_c[c]]
    return y, res


def kernel(H, edge_index, batch, W):
    y, _ = _run(H, edge_index, batch, W, n_graphs=256,
                trace=bool(os.environ.get("GCN_TRACE")))
    return y


# revision 28
# speedup vs baseline: 1.3098x; 1.3098x over previous
"""GCN layer with virtual node on 8 Trainium2 NeuronCores (Bass/Tile).

Reference computation (fp32):
    agg = segment_sum(H[src], dst, N)        # message passing
    out = H + agg
    vmean = segment_mean(out, batch, G)      # virtual node
    out = out + vmean[batch]
    y = relu(out @ W)

Distribution strategy (self-contained, hardcoded):
  - batch is sorted, G=256 graphs, 8 cores -> core c owns graphs
    [32c, 32c+32) == a contiguous node range (graph-aligned node sharding).
    Per-graph means never cross cores: no collectives needed.
  - Edges partitioned by owning core of dst (host-side index arithmetic).
    Within a core: 128-dst windows. Source rows are fetched from a DRAM fp16
    copy of H with gpsimd dma_gather (int16 indices -> 4 source "classes" of
    <=32768 rows). Gather calls are grouped over GW windows per class and
    spread round-robin over 4 SWDGE queues so descriptor generation runs on
    all four Q7 core pairs concurrently (measured ~3.2x vs single queue).
  - segment_sum via PE one-hot matmul in TRANSPOSED orientation:
    psum_wT[f, dst] += G_t^T @ R_t, where R_t[s, m] = (drel[s,t] == m) is
    built on DVE (is_equal vs iota, batched over a whole gather call), so
    out_T[f, node] accumulates in SBUF and the final y = out^T... matmul
    needs no PE transpose.
  - virtual node: per window w, transpose out_T_w back (PE+ident) and
    accumulate psum_sT[f, g] += out_w^T @ b_w with host-baked one-hot
    b_w[node, graph]; vmeanT = psum_sT * (1/count); VW = vmeanT^T @ W.
  - final: psum_y = out_T_w^T @ W + Bt_w^T @ VW (host-baked Bt[g, node]),
    relu on ACT, DMA out.
"""
import os
import numpy as np

from concourse import bacc, mybir
import concourse.tile as tile
from concourse.bass_utils import run_bass_kernel_spmd

P = 128
N_CORES = 8
D = 128
F32 = mybir.dt.float32
I16 = mybir.dt.int16
F16 = mybir.dt.float16
CLASS_SIZE = 32768  # int16 index reach for dma_gather
GW = 3              # windows per gather group (keeps calls under the
                    # 16KB/lane SWDGE ring: ~2300 idxs -> ~145 descs/lane)
NQ = 4              # SWDGE queues


def _ceil(a, b):
    return -(-a // b)


# ---------------------------------------------------------------------------
# host-side prep: pure index arithmetic / sharding metadata
# ---------------------------------------------------------------------------

def _prep(H, edge_index, batch, n_graphs):
    N = H.shape[0]
    src = np.asarray(edge_index[0], dtype=np.int64)
    dst = np.asarray(edge_index[1], dtype=np.int64)
    batch = np.asarray(batch, dtype=np.int64)
    gpc = n_graphs // N_CORES
    n_cls = _ceil(N, CLASS_SIZE)

    gstart = np.searchsorted(batch, np.arange(n_graphs + 1))
    core_start = gstart[::gpc]  # [N_CORES+1]
    counts = np.diff(gstart)

    node_core = (batch // gpc).astype(np.int64)
    ecore = node_core[dst]

    n_c = np.diff(core_start)
    NW = int(_ceil(n_c.max(), P))

    # per-core (w, k) counts and sorted edge runs
    cnt = np.zeros((N_CORES, NW, n_cls), dtype=np.int64)
    runs = []
    for c in range(N_CORES):
        m = ecore == c
        s_c, d_c = src[m], dst[m]
        dstl = d_c - core_start[c]
        w = dstl >> 7
        k = s_c // CLASS_SIZE
        key = w * n_cls + k
        # src-sorted within each (w, k) run: ascending gather addresses
        # give the SDMA engines better HBM row/bank locality.
        order = np.lexsort((s_c, key))
        s_c, dstl, key = s_c[order], dstl[order], key[order]
        cnt[c] = np.bincount(key, minlength=NW * n_cls).reshape(NW, n_cls)
        runs.append((s_c, dstl, key))

    tiles_wk = _ceil(cnt.max(axis=0), P)            # [NW, n_cls]
    tiles_wk[:, 0] = np.maximum(tiles_wk[:, 0], 1)  # every window has >=1 tile

    groups = [(lo, min(lo + GW, NW)) for lo in range(0, NW, GW)]
    # layout: for g: for k (rotated per group): for w in group -> tiles.
    # The class order rotates so the round-robin SWDGE queue assignment
    # (queue = issue order % 4, which must stay in lockstep with Tile's
    # DMASW sem-lane rotation) sees balanced work per queue despite the
    # small last class.
    calls = []  # (gi, k, call_t0, nt, [(w, wt0, ntw), ...])
    wt0 = np.zeros((NW, n_cls), dtype=np.int64)
    t = 0
    for gi, (lo, hi) in enumerate(groups):
        for j in range(n_cls):
            k = (j + gi) % n_cls
            c_t0 = t
            wl = []
            for w in range(lo, hi):
                wt0[w, k] = t
                wl.append((w, t, int(tiles_wk[w, k])))
                t += int(tiles_wk[w, k])
            calls.append((gi, k, c_t0, t - c_t0, wl))
    T = int(t)
    RT = max(nt for _, _, _, nt, _ in calls)

    params = dict(
        N=N, NW=NW, T=T, RT=RT, gpc=gpc, n_cls=n_cls,
        tiles=tuple(tuple(int(x) for x in row) for row in tiles_wk),
        groups=tuple(groups),
        cls_size=tuple(min(CLASS_SIZE, N - CLASS_SIZE * k)
                       for k in range(n_cls)),
    )

    in_maps = []
    h16 = np.ascontiguousarray(H, dtype=np.float16)
    for c in range(N_CORES):
        s_c, dstl, key = runs[c]
        idx_flat = np.full(T * P, -1, dtype=np.int64)
        drel = np.full(T * P, -1.0, dtype=np.float32)
        # slot of each edge: base slot of its (w,k) run + offset within run
        cnt_c = cnt[c].ravel()
        run_start = np.concatenate([[0], np.cumsum(cnt_c)])
        off_in_run = np.arange(len(key)) - run_start[key]
        slot = wt0.ravel()[key] * P + off_in_run
        idx_flat[slot] = s_c - (key % n_cls) * CLASS_SIZE
        drel[slot] = (dstl & 127).astype(np.float32)
        # interior pads (before the last real edge of each call) -> idx 0.
        # num_idxs_reg must equal the per-call count of non-negative idxs
        # (the SWDGE ring reserves space from the register while Q7
        # generates from the trailing-trimmed idx list -- a mismatch
        # corrupts the ring bookkeeping and wedges the device).
        cnt32 = np.zeros(len(calls), dtype=np.int32)
        for ci, (_, _, c_t0, nt, _) in enumerate(calls):
            blk = idx_flat[c_t0 * P:(c_t0 + nt) * P]
            real = np.nonzero(blk >= 0)[0]
            if len(real):
                last = int(real[-1])
            else:
                blk[0] = 0  # keep every call non-empty
                last = 0
            pad = blk[:last + 1] < 0
            blk[:last + 1][pad] = 0
            cnt32[ci] = last + 1
        # wrap idx per call region into [16, nt*8] blocks
        wrapped = np.full((16, T * P // 16), -1, dtype=np.int16)
        for _, _, c_t0, nt, _ in calls:
            blk = idx_flat[c_t0 * P:(c_t0 + nt) * P]
            wrapped[:, c_t0 * 8:(c_t0 + nt) * 8] = \
                blk.reshape(nt * 8, 16).T.astype(np.int16)
        wrapped128 = np.ascontiguousarray(np.tile(wrapped, (8, 1)))

        drel16 = np.ascontiguousarray(
            drel.reshape(T, P).T.astype(np.float16))

        nodes = int(n_c[c])
        hcT = np.zeros((P, NW * P), dtype=np.float32)
        hcT[:, :nodes] = np.asarray(H[core_start[c]:core_start[c] + nodes],
                                    dtype=np.float32).T
        br = np.full(NW * P, -1, dtype=np.int64)
        br[:nodes] = batch[core_start[c]:core_start[c] + nodes] - c * gpc
        ball = (br.reshape(NW, P).T[:, :, None]
                == np.arange(gpc)[None, None, :]).astype(np.float16)
        bt = (np.arange(gpc)[:, None] == br[None, :]).astype(np.float16)
        invc = np.broadcast_to(
            (1.0 / np.maximum(counts[c * gpc:(c + 1) * gpc], 1)
             ).astype(np.float32)[None, :], (P, gpc)).copy()

        in_maps.append({
            "h16": h16,
            "cnt32": np.ascontiguousarray(cnt32.reshape(1, -1)),
            "idx16": wrapped128,
            "drel": drel16,
            "hct": np.ascontiguousarray(hcT),
            "ball": np.ascontiguousarray(ball.reshape(P, NW * gpc)),
            "bt": np.ascontiguousarray(bt),
            "invc": invc,
        })
    return params, in_maps, n_c, core_start


def _consts(params, W):
    RT = params["RT"]
    iota_rep = np.broadcast_to(np.arange(P, dtype=np.float16),
                               (P, RT, P)).reshape(P, RT * P).copy()
    ident = np.eye(P, dtype=np.float16)
    return {"iota_rep": iota_rep, "ident": ident,
            "w16": np.ascontiguousarray(W, dtype=np.float16)}


# ---------------------------------------------------------------------------
# device kernel builder (SPMD: one program, per-core data)
# ---------------------------------------------------------------------------

def _build(params):
    NW, T, RT = params["NW"], params["T"], params["RT"]
    gpc, n_cls = params["gpc"], params["n_cls"]
    tiles = params["tiles"]
    groups = params["groups"]
    cls_size = params["cls_size"]
    N = params["N"]

    # reconstruct the call layout (same order as _prep)
    calls = []
    wt0 = {}
    t = 0
    for gi, (lo, hi) in enumerate(groups):
        for j in range(n_cls):
            k = (j + gi) % n_cls
            c_t0 = t
            wl = []
            for w in range(lo, hi):
                wt0[(w, k)] = t
                wl.append((w, t, tiles[w][k]))
                t += tiles[w][k]
            calls.append((gi, k, c_t0, t - c_t0, wl))
    assert t == T

    # max tiles per group (for idx/drel staging buffers)
    gt_span = []
    for gi in range(len(groups)):
        g_t0 = calls[gi * n_cls][2]
        last = calls[gi * n_cls + n_cls - 1]
        gt_span.append((g_t0, last[2] + last[3]))
    GT = max(b - a for a, b in gt_span)
    RTk = [max(nt for _, k2, _, nt, _ in calls if k2 == k)
           for k in range(n_cls)]

    nc = bacc.Bacc("TRN2", target_bir_lowering=False, debug=False,
                   num_devices=N_CORES, num_swdge_queues=NQ)
    h16_d = nc.dram_tensor("h16", [N, D], F16, kind="ExternalInput")
    cnt_d = nc.dram_tensor("cnt32", [1, len(calls)], mybir.dt.int32,
                           kind="ExternalInput")
    idx_d = nc.dram_tensor("idx16", [P, T * P // 16], I16,
                           kind="ExternalInput")
    drel_d = nc.dram_tensor("drel", [P, T], F16, kind="ExternalInput")
    hct_d = nc.dram_tensor("hct", [P, NW * P], F32, kind="ExternalInput")
    ball_d = nc.dram_tensor("ball", [P, NW * gpc], F16, kind="ExternalInput")
    bt_d = nc.dram_tensor("bt", [gpc, NW * P], F16, kind="ExternalInput")
    invc_d = nc.dram_tensor("invc", [P, gpc], F32, kind="ExternalInput")
    iota_d = nc.dram_tensor("iota_rep", [P, RT * P], F16,
                            kind="ExternalInput")
    ident_d = nc.dram_tensor("ident", [P, P], F16, kind="ExternalInput")
    w_d = nc.dram_tensor("w16", [P, D], F16, kind="ExternalInput")
    y_d = nc.dram_tensor("y", [NW * P, D], F32, kind="ExternalOutput")

    with tile.TileContext(nc) as tc:
        with tc.tile_pool(name="const", bufs=1) as cpool:
            iota_t = cpool.tile([P, RT, P], F16)
            nc.sync.dma_start(out=iota_t[:], in_=iota_d[:])
            cnt_t = cpool.tile([1, len(calls)], mybir.dt.int32)
            nc.sync.dma_start(out=cnt_t[:], in_=cnt_d[:])
            ident_t = cpool.tile([P, P], F16)
            nc.sync.dma_start(out=ident_t[:], in_=ident_d[:])
            w_t = cpool.tile([P, D], F16)
            nc.sync.dma_start(out=w_t[:], in_=w_d[:])
            invc_t = cpool.tile([P, gpc], F32)
            nc.sync.dma_start(out=invc_t[:], in_=invc_d[:])

            out_T = cpool.tile([P, NW, P], F16)
            vmeanT = cpool.tile([P, gpc], F16)
            vw16 = cpool.tile([gpc, D], F16)

            from contextlib import ExitStack
            with ExitStack() as stack:
                ep = stack.enter_context
                idxp = ep(tc.tile_pool(name="idxp", bufs=3))
                drelp = ep(tc.tile_pool(name="drelp", bufs=3))
                gpools = [ep(tc.tile_pool(name=f"g{k}", bufs=2))
                          for k in range(n_cls)]
                rpools = [ep(tc.tile_pool(name=f"r{k}", bufs=2))
                          for k in range(n_cls)]
                hcp = ep(tc.tile_pool(name="hcp", bufs=3))
                bp = ep(tc.tile_pool(name="bp", bufs=3))
                onp = ep(tc.tile_pool(name="onp", bufs=3))
                pwp = ep(tc.tile_pool(name="pw", bufs=4, space="PSUM"))
                ptrp = ep(tc.tile_pool(name="ptr", bufs=2, space="PSUM"))
                psp = ep(tc.tile_pool(name="ps", bufs=1, space="PSUM"))
                gcnt = ep(nc.gpsimd.register("gcnt"))
                psum_sT = psp.tile([P, gpc], F32, space="PSUM")
                n_gather = 0

                for gi, (lo, hi) in enumerate(groups):
                    g_t0, g_t1 = gt_span[gi]
                    gcols = g_t1 - g_t0
                    idx_t = idxp.tile([P, GT * 8], I16, tag="idx")
                    nc.sync.dma_start(
                        out=idx_t[:, :gcols * 8],
                        in_=idx_d[:, g_t0 * 8:g_t1 * 8])
                    drel_t = drelp.tile([P, GT], F16, tag="drel")
                    nc.sync.dma_start(
                        out=drel_t[:, :gcols],
                        in_=drel_d[:, g_t0:g_t1])

                    gts = {}
                    rts = {}
                    for j in range(n_cls):
                        _, k, c_t0, nt, _ = calls[gi * n_cls + j]
                        if nt == 0:
                            continue
                        g16 = gpools[k].tile([P, RTk[k], D], F16,
                                             tag=f"G{k}")
                        if gi < 2 or os.environ.get('GCN_SIM_MEMSET'):
                            # pad slots must be finite: 0 * NaN would poison
                            # the one-hot matmul. After the first rotation of
                            # the 2 buffers, stale content is old gathered
                            # rows (finite).
                            nc.vector.memset(g16[:], 0.0)
                        base = CLASS_SIZE * k
                        ci = gi * n_cls + j
                        nc.gpsimd.load(gcnt, cnt_t[0:1, ci:ci + 1])
                        nc.gpsimd.dma_gather(
                            out_ap=g16[:, :nt, :],
                            in_ap=h16_d[base:base + cls_size[k], :],
                            idxs_ap=idx_t[:, (c_t0 - g_t0) * 8:
                                          (c_t0 - g_t0 + nt) * 8],
                            num_idxs=nt * P,
                            num_idxs_reg=gcnt,
                            elem_size=D,
                            single_packet=False,
                            # queue stays in lockstep with Tile's DMASW
                            # sem-lane rotation (lane = issue order % 8,
                            # sems are locked to one SWDGE queue each)
                            queue_num=n_gather % NQ,
                        )
                        n_gather += 1
                        r16 = rpools[k].tile([P, RTk[k], P], F16,
                                             tag=f"R{k}")
                        nc.vector.tensor_tensor(
                            out=r16[:, :nt, :],
                            in0=drel_t[:, c_t0 - g_t0:c_t0 - g_t0 + nt
                                       ].to_broadcast([P, nt, P]),
                            in1=iota_t[:, :nt, :],
                            op=mybir.AluOpType.is_equal)
                        gts[k] = (g16, c_t0)
                        rts[k] = r16

                    for w in range(lo, hi):
                        psum_w = pwp.tile([P, P], F32, space="PSUM",
                                          tag="pw")
                        seq = []
                        for k in range(n_cls):
                            _, c_t0 = gts[k][0], gts[k][1]
                            for j in range(tiles[w][k]):
                                seq.append((k, wt0[(w, k)] - c_t0 + j))
                        for si, (k, j) in enumerate(seq):
                            nc.tensor.matmul(
                                psum_w[:], gts[k][0][:, j, :],
                                rts[k][:, j, :],
                                start=(si == 0), stop=(si == len(seq) - 1))
                        hc_t = hcp.tile([P, P], F32, tag="hc")
                        nc.sync.dma_start(out=hc_t[:],
                                          in_=hct_d[:, w * P:(w + 1) * P])
                        nc.vector.tensor_tensor(
                            out=out_T[:, w, :], in0=psum_w[:], in1=hc_t[:],
                            op=mybir.AluOpType.add)
                        # transpose back for the virtual-node segment sum
                        ptr_t = ptrp.tile([P, P], F16, space="PSUM",
                                          tag="ptr")
                        nc.tensor.transpose(ptr_t[:], out_T[:, w, :],
                                            ident_t[:])
                        on16 = onp.tile([P, P], F16, tag="on")
                        nc.vector.tensor_copy(on16[:], ptr_t[:])
                        b_t = bp.tile([P, gpc], F16, tag="b")
                        nc.sync.dma_start(
                            out=b_t[:],
                            in_=ball_d[:, w * gpc:(w + 1) * gpc])
                        nc.tensor.matmul(
                            psum_sT[:], on16[:], b_t[:],
                            start=(w == 0), stop=(w == NW - 1),
                            skip_group_check=True)

                nc.vector.tensor_tensor(
                    out=vmeanT[:], in0=psum_sT[:], in1=invc_t[:],
                    op=mybir.AluOpType.mult)

            with tc.tile_pool(name="p3", bufs=3) as p3, \
                 tc.tile_pool(name="pvw", bufs=1, space="PSUM") as pvwp, \
                 tc.tile_pool(name="py", bufs=2, space="PSUM") as pyp:
                psum_vw = pvwp.tile([gpc, D], F32, space="PSUM")
                nc.tensor.matmul(psum_vw[:], vmeanT[:], w_t[:],
                                 start=True, stop=True)
                nc.vector.tensor_copy(vw16[:], psum_vw[:])

                for w in range(NW):
                    bt_t = p3.tile([gpc, P], F16, tag="bt")
                    nc.sync.dma_start(out=bt_t[:],
                                      in_=bt_d[:, w * P:(w + 1) * P])
                    psum_y = pyp.tile([P, D], F32, space="PSUM", tag="py")
                    nc.tensor.matmul(psum_y[:], out_T[:, w, :], w_t[:],
                                     start=True, stop=False,
                                     skip_group_check=True)
                    nc.tensor.matmul(psum_y[:], bt_t[:], vw16[:],
                                     start=False, stop=True,
                                     skip_group_check=True)
                    y_t = p3.tile([P, D], F32, tag="Y")
                    nc.scalar.activation(y_t[:], psum_y[:],
                                         mybir.ActivationFunctionType.Relu)
                    nc.sync.dma_start(out=y_d[w * P:(w + 1) * P, :],
                                      in_=y_t[:])
    _finish_compile(nc)
    return nc


def _finish_compile(nc):
    nc.compile()
    # compile()'s tail passes (library-load insertion for the custom DMA
    # instructions) can reintroduce >1 sync wait per instruction, which the
    # TRN2 ISA rejects. Re-split and re-codegen.
    import bass_rust
    bass_rust.generate_event_semaphores(nc)
    nc.codegen_inst_isa_subclasses()


_BUILD_CACHE = {}


def _build_cached(params):
    key = tuple(sorted((k, str(v)) for k, v in params.items()))
    if key not in _BUILD_CACHE:
        _BUILD_CACHE[key] = _build(params)
    return _BUILD_CACHE[key]


def _run(H, edge_index, batch, W, n_graphs, trace=False):
    H = np.asarray(H)
    params, in_maps, n_c, core_start = _prep(H, edge_index, batch, n_graphs)
    consts = _consts(params, np.asarray(W))
    for m in in_maps:
        m.update(consts)
    nc = _build_cached(params)
    res = run_bass_kernel_spmd(nc, in_maps, list(range(N_CORES)), trace=trace)
    N = H.shape[0]
    y = np.empty((N, D), dtype=np.float32)
    for c in range(N_CORES):
        y[core_start[c]:core_start[c] + n_c[c]] = \
            res.results[c]["y"][:n_c[c]]
    return y, res


def kernel(H, edge_index, batch, W):
    y, _ = _run(H, edge_index, batch, W, n_graphs=256,
                trace=bool(os.environ.get("GCN_TRACE")))
    return y


# revision 29
# speedup vs baseline: 1.4130x; 1.0787x over previous
"""GCN layer with virtual node on 8 Trainium2 NeuronCores (Bass/Tile).

Reference computation (fp32):
    agg = segment_sum(H[src], dst, N)        # message passing
    out = H + agg
    vmean = segment_mean(out, batch, G)      # virtual node
    out = out + vmean[batch]
    y = relu(out @ W)

Distribution strategy (self-contained, hardcoded):
  - batch is sorted, G=256 graphs, 8 cores -> core c owns graphs
    [32c, 32c+32) == a contiguous node range (graph-aligned node sharding).
    Per-graph means never cross cores: no collectives needed.
  - Edges partitioned by owning core of dst (host-side index arithmetic).
    Within a core: 128-dst windows. Source rows are fetched from a DRAM fp16
    copy of H with gpsimd dma_gather (int16 indices -> 4 source "classes" of
    <=32768 rows). Gather calls are grouped over GW windows per class and
    spread round-robin over 4 SWDGE queues so descriptor generation runs on
    all four Q7 core pairs concurrently (measured ~3.2x vs single queue).
  - segment_sum via PE one-hot matmul in TRANSPOSED orientation:
    psum_wT[f, dst] += G_t^T @ R_t, where R_t[s, m] = (drel[s,t] == m) is
    built on DVE (is_equal vs iota, batched over a whole gather call), so
    out_T[f, node] accumulates in SBUF and the final y = out^T... matmul
    needs no PE transpose.
  - virtual node: per window w, transpose out_T_w back (PE+ident) and
    accumulate psum_sT[f, g] += out_w^T @ b_w with host-baked one-hot
    b_w[node, graph]; vmeanT = psum_sT * (1/count); VW = vmeanT^T @ W.
  - final: psum_y = out_T_w^T @ W + Bt_w^T @ VW (host-baked Bt[g, node]),
    relu on ACT, DMA out.
"""
import os
import numpy as np

from concourse import bacc, mybir
import concourse.tile as tile
from concourse.bass_utils import run_bass_kernel_spmd

P = 128
N_CORES = 8
D = 128
F32 = mybir.dt.float32
I16 = mybir.dt.int16
F16 = mybir.dt.float16
CLASS_SIZE = 32768  # int16 index reach for dma_gather
GW = 3              # windows per gather group (keeps calls under the
                    # 16KB/lane SWDGE ring: ~2300 idxs -> ~145 descs/lane)
NQ = 4              # SWDGE queues


def _ceil(a, b):
    return -(-a // b)


# ---------------------------------------------------------------------------
# host-side prep: pure index arithmetic / sharding metadata
# ---------------------------------------------------------------------------

def _prep(H, edge_index, batch, n_graphs):
    N = H.shape[0]
    src = np.asarray(edge_index[0], dtype=np.int64)
    dst = np.asarray(edge_index[1], dtype=np.int64)
    batch = np.asarray(batch, dtype=np.int64)
    gpc = n_graphs // N_CORES
    n_cls = _ceil(N, CLASS_SIZE)

    gstart = np.searchsorted(batch, np.arange(n_graphs + 1))
    core_start = gstart[::gpc]  # [N_CORES+1]
    counts = np.diff(gstart)

    node_core = (batch // gpc).astype(np.int64)
    ecore = node_core[dst]

    n_c = np.diff(core_start)
    NW = int(_ceil(n_c.max(), P))

    # per-core (w, k) counts and sorted edge runs
    cnt = np.zeros((N_CORES, NW, n_cls), dtype=np.int64)
    runs = []
    for c in range(N_CORES):
        m = ecore == c
        s_c, d_c = src[m], dst[m]
        dstl = d_c - core_start[c]
        w = dstl >> 7
        k = s_c // CLASS_SIZE
        key = w * n_cls + k
        # src-sorted within each (w, k) run: ascending gather addresses
        # give the SDMA engines better HBM row/bank locality.
        order = np.lexsort((s_c, key))
        s_c, dstl, key = s_c[order], dstl[order], key[order]
        cnt[c] = np.bincount(key, minlength=NW * n_cls).reshape(NW, n_cls)
        runs.append((s_c, dstl, key))

    tiles_wk = _ceil(cnt.max(axis=0), P)            # [NW, n_cls]
    tiles_wk[:, 0] = np.maximum(tiles_wk[:, 0], 1)  # every window has >=1 tile

    groups = [(lo, min(lo + GW, NW)) for lo in range(0, NW, GW)]
    # layout: for g: for k (rotated per group): for w in group -> tiles.
    # The class order rotates so the round-robin SWDGE queue assignment
    # (queue = issue order % 4, which must stay in lockstep with Tile's
    # DMASW sem-lane rotation) sees balanced work per queue despite the
    # small last class.
    calls = []  # (gi, k, call_t0, nt, [(w, wt0, ntw), ...])
    wt0 = np.zeros((NW, n_cls), dtype=np.int64)
    t = 0
    for gi, (lo, hi) in enumerate(groups):
        for j in range(n_cls):
            k = (j + gi) % n_cls
            c_t0 = t
            wl = []
            for w in range(lo, hi):
                wt0[w, k] = t
                wl.append((w, t, int(tiles_wk[w, k])))
                t += int(tiles_wk[w, k])
            calls.append((gi, k, c_t0, t - c_t0, wl))
    T = int(t)
    RT = max(nt for _, _, _, nt, _ in calls)

    params = dict(
        N=N, NW=NW, T=T, RT=RT, gpc=gpc, n_cls=n_cls,
        tiles=tuple(tuple(int(x) for x in row) for row in tiles_wk),
        groups=tuple(groups),
        cls_size=tuple(min(CLASS_SIZE, N - CLASS_SIZE * k)
                       for k in range(n_cls)),
    )

    in_maps = []
    h16 = np.ascontiguousarray(H, dtype=np.float16)
    for c in range(N_CORES):
        s_c, dstl, key = runs[c]
        idx_flat = np.full(T * P, -1, dtype=np.int64)
        drel = np.full(T * P, -1.0, dtype=np.float32)
        # slot of each edge: base slot of its (w,k) run + offset within run
        cnt_c = cnt[c].ravel()
        run_start = np.concatenate([[0], np.cumsum(cnt_c)])
        off_in_run = np.arange(len(key)) - run_start[key]
        slot = wt0.ravel()[key] * P + off_in_run
        idx_flat[slot] = s_c - (key % n_cls) * CLASS_SIZE
        drel[slot] = (dstl & 127).astype(np.float32)
        # interior pads (before the last real edge of each call) -> idx 0.
        # num_idxs_reg must equal the per-call count of non-negative idxs
        # (the SWDGE ring reserves space from the register while Q7
        # generates from the trailing-trimmed idx list -- a mismatch
        # corrupts the ring bookkeeping and wedges the device).
        cnt32 = np.zeros(len(calls), dtype=np.int32)
        for ci, (_, _, c_t0, nt, _) in enumerate(calls):
            blk = idx_flat[c_t0 * P:(c_t0 + nt) * P]
            real = np.nonzero(blk >= 0)[0]
            if len(real):
                last = int(real[-1])
            else:
                blk[0] = 0  # keep every call non-empty
                last = 0
            pad = blk[:last + 1] < 0
            blk[:last + 1][pad] = 0
            cnt32[ci] = last + 1
        # wrap idx per call region into [16, nt*8] blocks
        wrapped = np.full((16, T * P // 16), -1, dtype=np.int16)
        for _, _, c_t0, nt, _ in calls:
            blk = idx_flat[c_t0 * P:(c_t0 + nt) * P]
            wrapped[:, c_t0 * 8:(c_t0 + nt) * 8] = \
                blk.reshape(nt * 8, 16).T.astype(np.int16)
        wrapped128 = np.ascontiguousarray(np.tile(wrapped, (8, 1)))

        drel16 = np.ascontiguousarray(
            drel.reshape(T, P).T.astype(np.float16))

        nodes = int(n_c[c])
        hcT = np.zeros((P, NW * P), dtype=np.float32)
        hcT[:, :nodes] = np.asarray(H[core_start[c]:core_start[c] + nodes],
                                    dtype=np.float32).T
        br = np.full(NW * P, -1, dtype=np.int64)
        br[:nodes] = batch[core_start[c]:core_start[c] + nodes] - c * gpc
        ball = (br.reshape(NW, P).T[:, :, None]
                == np.arange(gpc)[None, None, :]).astype(np.float16)
        bt = (np.arange(gpc)[:, None] == br[None, :]).astype(np.float16)
        invc = np.broadcast_to(
            (1.0 / np.maximum(counts[c * gpc:(c + 1) * gpc], 1)
             ).astype(np.float32)[None, :], (P, gpc)).copy()

        in_maps.append({
            "h16": h16,
            "cnt32": np.ascontiguousarray(cnt32.reshape(1, -1)),
            "idx16": wrapped128,
            "drel": drel16,
            "hct": np.ascontiguousarray(hcT),
            "ball": np.ascontiguousarray(ball.reshape(P, NW * gpc)),
            "bt": np.ascontiguousarray(bt),
            "invc": invc,
        })
    return params, in_maps, n_c, core_start


def _consts(params, W):
    RT = params["RT"]
    iota_rep = np.broadcast_to(np.arange(P, dtype=np.float16),
                               (P, RT, P)).reshape(P, RT * P).copy()
    ident = np.eye(P, dtype=np.float16)
    return {"iota_rep": iota_rep, "ident": ident,
            "w16": np.ascontiguousarray(W, dtype=np.float16)}


# ---------------------------------------------------------------------------
# device kernel builder (SPMD: one program, per-core data)
# ---------------------------------------------------------------------------

def _build(params):
    NW, T, RT = params["NW"], params["T"], params["RT"]
    gpc, n_cls = params["gpc"], params["n_cls"]
    tiles = params["tiles"]
    groups = params["groups"]
    cls_size = params["cls_size"]
    N = params["N"]

    # reconstruct the call layout (same order as _prep)
    calls = []
    wt0 = {}
    t = 0
    for gi, (lo, hi) in enumerate(groups):
        for j in range(n_cls):
            k = (j + gi) % n_cls
            c_t0 = t
            wl = []
            for w in range(lo, hi):
                wt0[(w, k)] = t
                wl.append((w, t, tiles[w][k]))
                t += tiles[w][k]
            calls.append((gi, k, c_t0, t - c_t0, wl))
    assert t == T

    # max tiles per group (for idx/drel staging buffers)
    gt_span = []
    for gi in range(len(groups)):
        g_t0 = calls[gi * n_cls][2]
        last = calls[gi * n_cls + n_cls - 1]
        gt_span.append((g_t0, last[2] + last[3]))
    GT = max(b - a for a, b in gt_span)
    RTk = [max(nt for _, k2, _, nt, _ in calls if k2 == k)
           for k in range(n_cls)]

    nc = bacc.Bacc("TRN2", target_bir_lowering=False, debug=False,
                   num_devices=N_CORES, num_swdge_queues=NQ)
    h16_d = nc.dram_tensor("h16", [N, D], F16, kind="ExternalInput")
    cnt_d = nc.dram_tensor("cnt32", [1, len(calls)], mybir.dt.int32,
                           kind="ExternalInput")
    idx_d = nc.dram_tensor("idx16", [P, T * P // 16], I16,
                           kind="ExternalInput")
    drel_d = nc.dram_tensor("drel", [P, T], F16, kind="ExternalInput")
    hct_d = nc.dram_tensor("hct", [P, NW * P], F32, kind="ExternalInput")
    ball_d = nc.dram_tensor("ball", [P, NW * gpc], F16, kind="ExternalInput")
    bt_d = nc.dram_tensor("bt", [gpc, NW * P], F16, kind="ExternalInput")
    invc_d = nc.dram_tensor("invc", [P, gpc], F32, kind="ExternalInput")
    iota_d = nc.dram_tensor("iota_rep", [P, RT * P], F16,
                            kind="ExternalInput")
    ident_d = nc.dram_tensor("ident", [P, P], F16, kind="ExternalInput")
    w_d = nc.dram_tensor("w16", [P, D], F16, kind="ExternalInput")
    y_d = nc.dram_tensor("y", [NW * P, D], F32, kind="ExternalOutput")

    with tile.TileContext(nc) as tc:
        with tc.tile_pool(name="const", bufs=1) as cpool:
            iota_t = cpool.tile([P, RT, P], F16)
            nc.sync.dma_start(out=iota_t[:], in_=iota_d[:])
            cnt_t = cpool.tile([1, len(calls)], mybir.dt.int32)
            nc.sync.dma_start(out=cnt_t[:], in_=cnt_d[:])
            ident_t = cpool.tile([P, P], F16)
            nc.sync.dma_start(out=ident_t[:], in_=ident_d[:])
            w_t = cpool.tile([P, D], F16)
            nc.sync.dma_start(out=w_t[:], in_=w_d[:])
            invc_t = cpool.tile([P, gpc], F32)
            nc.sync.dma_start(out=invc_t[:], in_=invc_d[:])

            out_T = cpool.tile([P, NW, P], F16)
            vmeanT = cpool.tile([P, gpc], F16)
            vw16 = cpool.tile([gpc, D], F16)

            from contextlib import ExitStack
            with ExitStack() as stack:
                ep = stack.enter_context
                idxp = ep(tc.tile_pool(name="idxp", bufs=3))
                drelp = ep(tc.tile_pool(name="drelp", bufs=3))
                gpools = [ep(tc.tile_pool(name=f"g{k}", bufs=3))
                          for k in range(n_cls)]
                rpools = [ep(tc.tile_pool(name=f"r{k}", bufs=3))
                          for k in range(n_cls)]
                hcp = ep(tc.tile_pool(name="hcp", bufs=3))
                bp = ep(tc.tile_pool(name="bp", bufs=3))
                onp = ep(tc.tile_pool(name="onp", bufs=3))
                pwp = ep(tc.tile_pool(name="pw", bufs=4, space="PSUM"))
                ptrp = ep(tc.tile_pool(name="ptr", bufs=2, space="PSUM"))
                psp = ep(tc.tile_pool(name="ps", bufs=1, space="PSUM"))
                gcnt = ep(nc.gpsimd.register("gcnt"))
                psum_sT = psp.tile([P, gpc], F32, space="PSUM")
                n_gather = 0

                for gi, (lo, hi) in enumerate(groups):
                    g_t0, g_t1 = gt_span[gi]
                    gcols = g_t1 - g_t0
                    idx_t = idxp.tile([P, GT * 8], I16, tag="idx")
                    nc.sync.dma_start(
                        out=idx_t[:, :gcols * 8],
                        in_=idx_d[:, g_t0 * 8:g_t1 * 8])
                    drel_t = drelp.tile([P, GT], F16, tag="drel")
                    nc.sync.dma_start(
                        out=drel_t[:, :gcols],
                        in_=drel_d[:, g_t0:g_t1])

                    gts = {}
                    rts = {}
                    for j in range(n_cls):
                        _, k, c_t0, nt, _ = calls[gi * n_cls + j]
                        if nt == 0:
                            continue
                        g16 = gpools[k].tile([P, RTk[k], D], F16,
                                             tag=f"G{k}")
                        if gi < 3 or os.environ.get('GCN_SIM_MEMSET'):
                            # pad slots must be finite: 0 * NaN would poison
                            # the one-hot matmul. After the first rotation of
                            # the 3 buffers, stale content is old gathered
                            # rows (finite).
                            nc.vector.memset(g16[:], 0.0)
                        base = CLASS_SIZE * k
                        ci = gi * n_cls + j
                        nc.gpsimd.load(gcnt, cnt_t[0:1, ci:ci + 1])
                        nc.gpsimd.dma_gather(
                            out_ap=g16[:, :nt, :],
                            in_ap=h16_d[base:base + cls_size[k], :],
                            idxs_ap=idx_t[:, (c_t0 - g_t0) * 8:
                                          (c_t0 - g_t0 + nt) * 8],
                            num_idxs=nt * P,
                            num_idxs_reg=gcnt,
                            elem_size=D,
                            single_packet=False,
                            # queue stays in lockstep with Tile's DMASW
                            # sem-lane rotation (lane = issue order % 8,
                            # sems are locked to one SWDGE queue each)
                            queue_num=n_gather % NQ,
                        )
                        n_gather += 1
                        r16 = rpools[k].tile([P, RTk[k], P], F16,
                                             tag=f"R{k}")
                        nc.vector.tensor_tensor(
                            out=r16[:, :nt, :],
                            in0=drel_t[:, c_t0 - g_t0:c_t0 - g_t0 + nt
                                       ].to_broadcast([P, nt, P]),
                            in1=iota_t[:, :nt, :],
                            op=mybir.AluOpType.is_equal)
                        gts[k] = (g16, c_t0)
                        rts[k] = r16

                    for w in range(lo, hi):
                        psum_w = pwp.tile([P, P], F32, space="PSUM",
                                          tag="pw")
                        seq = []
                        for k in range(n_cls):
                            _, c_t0 = gts[k][0], gts[k][1]
                            for j in range(tiles[w][k]):
                                seq.append((k, wt0[(w, k)] - c_t0 + j))
                        for si, (k, j) in enumerate(seq):
                            nc.tensor.matmul(
                                psum_w[:], gts[k][0][:, j, :],
                                rts[k][:, j, :],
                                start=(si == 0), stop=(si == len(seq) - 1))
                        hc_t = hcp.tile([P, P], F32, tag="hc")
                        nc.sync.dma_start(out=hc_t[:],
                                          in_=hct_d[:, w * P:(w + 1) * P])
                        nc.vector.tensor_tensor(
                            out=out_T[:, w, :], in0=psum_w[:], in1=hc_t[:],
                            op=mybir.AluOpType.add)
                        # transpose back for the virtual-node segment sum
                        ptr_t = ptrp.tile([P, P], F16, space="PSUM",
                                          tag="ptr")
                        nc.tensor.transpose(ptr_t[:], out_T[:, w, :],
                                            ident_t[:])
                        on16 = onp.tile([P, P], F16, tag="on")
                        nc.vector.tensor_copy(on16[:], ptr_t[:])
                        b_t = bp.tile([P, gpc], F16, tag="b")
                        nc.sync.dma_start(
                            out=b_t[:],
                            in_=ball_d[:, w * gpc:(w + 1) * gpc])
                        nc.tensor.matmul(
                            psum_sT[:], on16[:], b_t[:],
                            start=(w == 0), stop=(w == NW - 1),
                            skip_group_check=True)

                nc.vector.tensor_tensor(
                    out=vmeanT[:], in0=psum_sT[:], in1=invc_t[:],
                    op=mybir.AluOpType.mult)

            with tc.tile_pool(name="p3", bufs=3) as p3, \
                 tc.tile_pool(name="pvw", bufs=1, space="PSUM") as pvwp, \
                 tc.tile_pool(name="py", bufs=2, space="PSUM") as pyp:
                psum_vw = pvwp.tile([gpc, D], F32, space="PSUM")
                nc.tensor.matmul(psum_vw[:], vmeanT[:], w_t[:],
                                 start=True, stop=True)
                nc.vector.tensor_copy(vw16[:], psum_vw[:])

                for w in range(NW):
                    bt_t = p3.tile([gpc, P], F16, tag="bt")
                    nc.sync.dma_start(out=bt_t[:],
                                      in_=bt_d[:, w * P:(w + 1) * P])
                    psum_y = pyp.tile([P, D], F32, space="PSUM", tag="py")
                    nc.tensor.matmul(psum_y[:], out_T[:, w, :], w_t[:],
                                     start=True, stop=False,
                                     skip_group_check=True)
                    nc.tensor.matmul(psum_y[:], bt_t[:], vw16[:],
                                     start=False, stop=True,
                                     skip_group_check=True)
                    y_t = p3.tile([P, D], F32, tag="Y")
                    nc.scalar.activation(y_t[:], psum_y[:],
                                         mybir.ActivationFunctionType.Relu)
                    nc.sync.dma_start(out=y_d[w * P:(w + 1) * P, :],
                                      in_=y_t[:])
    _finish_compile(nc)
    return nc


def _finish_compile(nc):
    nc.compile()
    # compile()'s tail passes (library-load insertion for the custom DMA
    # instructions) can reintroduce >1 sync wait per instruction, which the
    # TRN2 ISA rejects. Re-split and re-codegen.
    import bass_rust
    bass_rust.generate_event_semaphores(nc)
    nc.codegen_inst_isa_subclasses()


_BUILD_CACHE = {}


def _build_cached(params):
    key = tuple(sorted((k, str(v)) for k, v in params.items()))
    if key not in _BUILD_CACHE:
        _BUILD_CACHE[key] = _build(params)
    return _BUILD_CACHE[key]


def _run(H, edge_index, batch, W, n_graphs, trace=False):
    H = np.asarray(H)
    params, in_maps, n_c, core_start = _prep(H, edge_index, batch, n_graphs)
    consts = _consts(params, np.asarray(W))
    for m in in_maps:
        m.update(consts)
    nc = _build_cached(params)
    res = run_bass_kernel_spmd(nc, in_maps, list(range(N_CORES)), trace=trace)
    N = H.shape[0]
    y = np.empty((N, D), dtype=np.float32)
    for c in range(N_CORES):
        y[core_start[c]:core_start[c] + n_c[c]] = \
            res.results[c]["y"][:n_c[c]]
    return y, res


def kernel(H, edge_index, batch, W):
    y, _ = _run(H, edge_index, batch, W, n_graphs=256,
                trace=bool(os.environ.get("GCN_TRACE")))
    return y


# revision 32
# speedup vs baseline: 1.4206x; 1.0054x over previous
"""GCN layer with virtual node on 8 Trainium2 NeuronCores (Bass/Tile).

Reference computation (fp32):
    agg = segment_sum(H[src], dst, N)        # message passing
    out = H + agg
    vmean = segment_mean(out, batch, G)      # virtual node
    out = out + vmean[batch]
    y = relu(out @ W)

Distribution strategy (self-contained, hardcoded):
  - batch is sorted, G=256 graphs, 8 cores -> core c owns graphs
    [32c, 32c+32) == a contiguous node range (graph-aligned node sharding).
    Per-graph means never cross cores: no collectives needed.
  - Edges partitioned by owning core of dst (host-side index arithmetic).
    Within a core: 128-dst windows. Source rows are fetched from a DRAM fp16
    copy of H with gpsimd dma_gather (int16 indices -> 4 source "classes" of
    <=32768 rows). Gather calls are grouped over GW windows per class and
    spread round-robin over 4 SWDGE queues so descriptor generation runs on
    all four Q7 core pairs concurrently (measured ~3.2x vs single queue).
  - segment_sum via PE one-hot matmul in TRANSPOSED orientation:
    psum_wT[f, dst] += G_t^T @ R_t, where R_t[s, m] = (drel[s,t] == m) is
    built on DVE (is_equal vs iota, batched over a whole gather call), so
    out_T[f, node] accumulates in SBUF and the final y = out^T... matmul
    needs no PE transpose.
  - virtual node: per window w, transpose out_T_w back (PE+ident) and
    accumulate psum_sT[f, g] += out_w^T @ b_w with host-baked one-hot
    b_w[node, graph]; vmeanT = psum_sT * (1/count); VW = vmeanT^T @ W.
  - final: psum_y = out_T_w^T @ W + Bt_w^T @ VW (host-baked Bt[g, node]),
    relu on ACT, DMA out.
"""
import os
import numpy as np

from concourse import bacc, mybir
import concourse.tile as tile
from concourse.bass_utils import run_bass_kernel_spmd

P = 128
N_CORES = 8
D = 128
F32 = mybir.dt.float32
I16 = mybir.dt.int16
F16 = mybir.dt.float16
CLASS_SIZE = 32768  # int16 index reach for dma_gather
GW = 3              # windows per gather group (keeps calls under the
                    # 16KB/lane SWDGE ring: ~2300 idxs -> ~145 descs/lane)
NQ = 4              # SWDGE queues


def _ceil(a, b):
    return -(-a // b)


# ---------------------------------------------------------------------------
# host-side prep: pure index arithmetic / sharding metadata
# ---------------------------------------------------------------------------

def _prep(H, edge_index, batch, n_graphs):
    N = H.shape[0]
    src = np.asarray(edge_index[0], dtype=np.int64)
    dst = np.asarray(edge_index[1], dtype=np.int64)
    batch = np.asarray(batch, dtype=np.int64)
    gpc = n_graphs // N_CORES
    n_cls = _ceil(N, CLASS_SIZE)

    gstart = np.searchsorted(batch, np.arange(n_graphs + 1))
    core_start = gstart[::gpc]  # [N_CORES+1]
    counts = np.diff(gstart)

    node_core = (batch // gpc).astype(np.int64)
    ecore = node_core[dst]

    n_c = np.diff(core_start)
    NW = int(_ceil(n_c.max(), P))

    # per-core (w, k) counts and sorted edge runs
    cnt = np.zeros((N_CORES, NW, n_cls), dtype=np.int64)
    runs = []
    for c in range(N_CORES):
        m = ecore == c
        s_c, d_c = src[m], dst[m]
        dstl = d_c - core_start[c]
        w = dstl >> 7
        k = s_c // CLASS_SIZE
        key = w * n_cls + k
        # src-sorted within each (w, k) run: ascending gather addresses
        # give the SDMA engines better HBM row/bank locality.
        order = np.lexsort((s_c, key))
        s_c, dstl, key = s_c[order], dstl[order], key[order]
        cnt[c] = np.bincount(key, minlength=NW * n_cls).reshape(NW, n_cls)
        runs.append((s_c, dstl, key))

    tiles_wk = _ceil(cnt.max(axis=0), P)            # [NW, n_cls]
    tiles_wk[:, 0] = np.maximum(tiles_wk[:, 0], 1)  # every window has >=1 tile

    groups = [(lo, min(lo + GW, NW)) for lo in range(0, NW, GW)]
    # layout: for g: for k (rotated per group): for w in group -> tiles.
    # The class order rotates so the round-robin SWDGE queue assignment
    # (queue = issue order % 4, which must stay in lockstep with Tile's
    # DMASW sem-lane rotation) sees balanced work per queue despite the
    # small last class.
    calls = []  # (gi, k, call_t0, nt, [(w, wt0, ntw), ...])
    wt0 = np.zeros((NW, n_cls), dtype=np.int64)
    t = 0
    for gi, (lo, hi) in enumerate(groups):
        for j in range(n_cls):
            k = (j + gi) % n_cls
            c_t0 = t
            wl = []
            for w in range(lo, hi):
                wt0[w, k] = t
                wl.append((w, t, int(tiles_wk[w, k])))
                t += int(tiles_wk[w, k])
            calls.append((gi, k, c_t0, t - c_t0, wl))
    T = int(t)
    RT = max(nt for _, _, _, nt, _ in calls)

    params = dict(
        N=N, NW=NW, T=T, RT=RT, gpc=gpc, n_cls=n_cls,
        tiles=tuple(tuple(int(x) for x in row) for row in tiles_wk),
        groups=tuple(groups),
        cls_size=tuple(min(CLASS_SIZE, N - CLASS_SIZE * k)
                       for k in range(n_cls)),
    )

    in_maps = []
    h16 = np.ascontiguousarray(H, dtype=np.float16)
    for c in range(N_CORES):
        s_c, dstl, key = runs[c]
        idx_flat = np.full(T * P, -1, dtype=np.int64)
        drel = np.full(T * P, -1.0, dtype=np.float32)
        # slot of each edge: base slot of its (w,k) run + offset within run
        cnt_c = cnt[c].ravel()
        run_start = np.concatenate([[0], np.cumsum(cnt_c)])
        off_in_run = np.arange(len(key)) - run_start[key]
        slot = wt0.ravel()[key] * P + off_in_run
        idx_flat[slot] = s_c - (key % n_cls) * CLASS_SIZE
        drel[slot] = (dstl & 127).astype(np.float32)
        # interior pads (before the last real edge of each call) -> idx 0.
        # num_idxs_reg must equal the per-call count of non-negative idxs
        # (the SWDGE ring reserves space from the register while Q7
        # generates from the trailing-trimmed idx list -- a mismatch
        # corrupts the ring bookkeeping and wedges the device).
        cnt32 = np.zeros(len(calls), dtype=np.int32)
        for ci, (_, _, c_t0, nt, _) in enumerate(calls):
            blk = idx_flat[c_t0 * P:(c_t0 + nt) * P]
            real = np.nonzero(blk >= 0)[0]
            if len(real):
                last = int(real[-1])
            else:
                blk[0] = 0  # keep every call non-empty
                last = 0
            pad = blk[:last + 1] < 0
            blk[:last + 1][pad] = 0
            cnt32[ci] = last + 1
        # wrap idx per call region into [16, nt*8] blocks
        wrapped = np.full((16, T * P // 16), -1, dtype=np.int16)
        for _, _, c_t0, nt, _ in calls:
            blk = idx_flat[c_t0 * P:(c_t0 + nt) * P]
            wrapped[:, c_t0 * 8:(c_t0 + nt) * 8] = \
                blk.reshape(nt * 8, 16).T.astype(np.int16)
        wrapped128 = np.ascontiguousarray(np.tile(wrapped, (8, 1)))

        drel16 = np.ascontiguousarray(
            drel.reshape(T, P).T.astype(np.float16))

        nodes = int(n_c[c])
        hcT = np.zeros((P, NW * P), dtype=np.float32)
        hcT[:, :nodes] = np.asarray(H[core_start[c]:core_start[c] + nodes],
                                    dtype=np.float32).T
        br = np.full(NW * P, -1, dtype=np.int64)
        br[:nodes] = batch[core_start[c]:core_start[c] + nodes] - c * gpc
        ball = (br.reshape(NW, P).T[:, :, None]
                == np.arange(gpc)[None, None, :]).astype(np.float16)
        bt = (np.arange(gpc)[:, None] == br[None, :]).astype(np.float16)
        invc = np.broadcast_to(
            (1.0 / np.maximum(counts[c * gpc:(c + 1) * gpc], 1)
             ).astype(np.float32)[None, :], (P, gpc)).copy()

        in_maps.append({
            "h16": h16,
            "cnt32": np.ascontiguousarray(cnt32.reshape(1, -1)),
            "idx16": wrapped128,
            "drel": drel16,
            "hct": np.ascontiguousarray(hcT),
            "ball": np.ascontiguousarray(ball.reshape(P, NW * gpc)),
            "bt": np.ascontiguousarray(bt),
            "invc": invc,
        })
    return params, in_maps, n_c, core_start


def _consts(params, W):
    RT = params["RT"]
    iota_rep = np.broadcast_to(np.arange(P, dtype=np.float16),
                               (P, RT, P)).reshape(P, RT * P).copy()
    ident = np.eye(P, dtype=np.float16)
    return {"iota_rep": iota_rep, "ident": ident,
            "w16": np.ascontiguousarray(W, dtype=np.float16)}


# ---------------------------------------------------------------------------
# device kernel builder (SPMD: one program, per-core data)
# ---------------------------------------------------------------------------

def _build(params):
    NW, T, RT = params["NW"], params["T"], params["RT"]
    gpc, n_cls = params["gpc"], params["n_cls"]
    tiles = params["tiles"]
    groups = params["groups"]
    cls_size = params["cls_size"]
    N = params["N"]

    # reconstruct the call layout (same order as _prep)
    calls = []
    wt0 = {}
    t = 0
    for gi, (lo, hi) in enumerate(groups):
        for j in range(n_cls):
            k = (j + gi) % n_cls
            c_t0 = t
            wl = []
            for w in range(lo, hi):
                wt0[(w, k)] = t
                wl.append((w, t, tiles[w][k]))
                t += tiles[w][k]
            calls.append((gi, k, c_t0, t - c_t0, wl))
    assert t == T

    # max tiles per group (for idx/drel staging buffers)
    gt_span = []
    for gi in range(len(groups)):
        g_t0 = calls[gi * n_cls][2]
        last = calls[gi * n_cls + n_cls - 1]
        gt_span.append((g_t0, last[2] + last[3]))
    GT = max(b - a for a, b in gt_span)
    RTk = [max(nt for _, k2, _, nt, _ in calls if k2 == k)
           for k in range(n_cls)]

    nc = bacc.Bacc("TRN2", target_bir_lowering=False, debug=False,
                   num_devices=N_CORES, num_swdge_queues=NQ)
    h16_d = nc.dram_tensor("h16", [N, D], F16, kind="ExternalInput")
    cnt_d = nc.dram_tensor("cnt32", [1, len(calls)], mybir.dt.int32,
                           kind="ExternalInput")
    idx_d = nc.dram_tensor("idx16", [P, T * P // 16], I16,
                           kind="ExternalInput")
    drel_d = nc.dram_tensor("drel", [P, T], F16, kind="ExternalInput")
    hct_d = nc.dram_tensor("hct", [P, NW * P], F32, kind="ExternalInput")
    ball_d = nc.dram_tensor("ball", [P, NW * gpc], F16, kind="ExternalInput")
    bt_d = nc.dram_tensor("bt", [gpc, NW * P], F16, kind="ExternalInput")
    invc_d = nc.dram_tensor("invc", [P, gpc], F32, kind="ExternalInput")
    iota_d = nc.dram_tensor("iota_rep", [P, RT * P], F16,
                            kind="ExternalInput")
    ident_d = nc.dram_tensor("ident", [P, P], F16, kind="ExternalInput")
    w_d = nc.dram_tensor("w16", [P, D], F16, kind="ExternalInput")
    y_d = nc.dram_tensor("y", [NW * P, D], F32, kind="ExternalOutput")

    with tile.TileContext(nc) as tc:
        with tc.tile_pool(name="const", bufs=1) as cpool:
            iota_t = cpool.tile([P, RT, P], F16)
            nc.sync.dma_start(out=iota_t[:], in_=iota_d[:])
            cnt_t = cpool.tile([1, len(calls)], mybir.dt.int32)
            nc.sync.dma_start(out=cnt_t[:], in_=cnt_d[:])
            ident_t = cpool.tile([P, P], F16)
            nc.sync.dma_start(out=ident_t[:], in_=ident_d[:])
            w_t = cpool.tile([P, D], F16)
            nc.sync.dma_start(out=w_t[:], in_=w_d[:])
            invc_t = cpool.tile([P, gpc], F32)
            nc.sync.dma_start(out=invc_t[:], in_=invc_d[:])

            out_T = cpool.tile([P, NW, P], F16)
            vmeanT = cpool.tile([P, gpc], F16)
            vw16 = cpool.tile([gpc, D], F16)

            from contextlib import ExitStack
            with ExitStack() as stack:
                ep = stack.enter_context
                idxp = ep(tc.tile_pool(name="idxp", bufs=3))
                drelp = ep(tc.tile_pool(name="drelp", bufs=3))
                gpools = [ep(tc.tile_pool(name=f"g{k}", bufs=3))
                          for k in range(n_cls)]
                rpools = [ep(tc.tile_pool(name=f"r{k}", bufs=3))
                          for k in range(n_cls)]
                hcp = ep(tc.tile_pool(name="hcp", bufs=3))
                bp = ep(tc.tile_pool(name="bp", bufs=3))
                onp = ep(tc.tile_pool(name="onp", bufs=3))
                pwp = ep(tc.tile_pool(name="pw", bufs=5, space="PSUM"))
                ptrp = ep(tc.tile_pool(name="ptr", bufs=2, space="PSUM"))
                psp = ep(tc.tile_pool(name="ps", bufs=1, space="PSUM"))
                gcnt = ep(nc.gpsimd.register("gcnt"))
                psum_sT = psp.tile([P, gpc], F32, space="PSUM")
                n_gather = 0

                for gi, (lo, hi) in enumerate(groups):
                    g_t0, g_t1 = gt_span[gi]
                    gcols = g_t1 - g_t0
                    idx_t = idxp.tile([P, GT * 8], I16, tag="idx")
                    nc.sync.dma_start(
                        out=idx_t[:, :gcols * 8],
                        in_=idx_d[:, g_t0 * 8:g_t1 * 8])
                    drel_t = drelp.tile([P, GT], F16, tag="drel")
                    nc.sync.dma_start(
                        out=drel_t[:, :gcols],
                        in_=drel_d[:, g_t0:g_t1])

                    gts = {}
                    rts = {}
                    for j in range(n_cls):
                        _, k, c_t0, nt, _ = calls[gi * n_cls + j]
                        if nt == 0:
                            continue
                        g16 = gpools[k].tile([P, RTk[k], D], F16,
                                             tag=f"G{k}")
                        if gi < 3 or os.environ.get('GCN_SIM_MEMSET'):
                            # pad slots must be finite: 0 * NaN would poison
                            # the one-hot matmul. After the first rotation of
                            # the 3 buffers, stale content is old gathered
                            # rows (finite).
                            nc.vector.memset(g16[:], 0.0)
                        base = CLASS_SIZE * k
                        ci = gi * n_cls + j
                        nc.gpsimd.load(gcnt, cnt_t[0:1, ci:ci + 1])
                        nc.gpsimd.dma_gather(
                            out_ap=g16[:, :nt, :],
                            in_ap=h16_d[base:base + cls_size[k], :],
                            idxs_ap=idx_t[:, (c_t0 - g_t0) * 8:
                                          (c_t0 - g_t0 + nt) * 8],
                            num_idxs=nt * P,
                            num_idxs_reg=gcnt,
                            elem_size=D,
                            single_packet=False,
                            # queue stays in lockstep with Tile's DMASW
                            # sem-lane rotation (lane = issue order % 8,
                            # sems are locked to one SWDGE queue each)
                            queue_num=n_gather % NQ,
                        )
                        n_gather += 1
                        r16 = rpools[k].tile([P, RTk[k], P], F16,
                                             tag=f"R{k}")
                        nc.vector.tensor_tensor(
                            out=r16[:, :nt, :],
                            in0=drel_t[:, c_t0 - g_t0:c_t0 - g_t0 + nt
                                       ].to_broadcast([P, nt, P]),
                            in1=iota_t[:, :nt, :],
                            op=mybir.AluOpType.is_equal)
                        gts[k] = (g16, c_t0)
                        rts[k] = r16

                    # class-major matmul emission: each class's matmuls
                    # run as soon as ITS gather lands, releasing that G/R
                    # buffer without waiting for the group's slowest call
                    psum_ws = {}
                    nseq = {w: sum(tiles[w][k2] for k2 in range(n_cls))
                            for w in range(lo, hi)}
                    done = {w: 0 for w in range(lo, hi)}
                    for j2 in range(n_cls):
                        _, k, c_t0, nt, _ = calls[gi * n_cls + j2]
                        if nt == 0:
                            continue
                        for w in range(lo, hi):
                            if tiles[w][k] == 0:
                                continue
                            if w not in psum_ws:
                                psum_w = pwp.tile([P, P], F32,
                                                  space="PSUM", tag="pw")
                                psum_ws[w] = psum_w
                            for j in range(tiles[w][k]):
                                nc.tensor.matmul(
                                    psum_ws[w][:], gts[k][0][:, j + wt0[(w, k)] - c_t0, :],
                                    rts[k][:, j + wt0[(w, k)] - c_t0, :],
                                    start=(done[w] == 0 and j == 0),
                                    stop=(done[w] + j + 1 == nseq[w]),
                                    skip_group_check=True)
                            done[w] += tiles[w][k]
                    for w in range(lo, hi):
                        psum_w = psum_ws[w]
                        hc_t = hcp.tile([P, P], F32, tag="hc")
                        nc.sync.dma_start(out=hc_t[:],
                                          in_=hct_d[:, w * P:(w + 1) * P])
                        nc.vector.tensor_tensor(
                            out=out_T[:, w, :], in0=psum_w[:], in1=hc_t[:],
                            op=mybir.AluOpType.add)
                        # transpose back for the virtual-node segment sum
                        ptr_t = ptrp.tile([P, P], F16, space="PSUM",
                                          tag="ptr")
                        nc.tensor.transpose(ptr_t[:], out_T[:, w, :],
                                            ident_t[:])
                        on16 = onp.tile([P, P], F16, tag="on")
                        nc.vector.tensor_copy(on16[:], ptr_t[:])
                        b_t = bp.tile([P, gpc], F16, tag="b")
                        nc.sync.dma_start(
                            out=b_t[:],
                            in_=ball_d[:, w * gpc:(w + 1) * gpc])
                        nc.tensor.matmul(
                            psum_sT[:], on16[:], b_t[:],
                            start=(w == 0), stop=(w == NW - 1),
                            skip_group_check=True)

                nc.vector.tensor_tensor(
                    out=vmeanT[:], in0=psum_sT[:], in1=invc_t[:],
                    op=mybir.AluOpType.mult)

            with tc.tile_pool(name="p3", bufs=3) as p3, \
                 tc.tile_pool(name="pvw", bufs=1, space="PSUM") as pvwp, \
                 tc.tile_pool(name="py", bufs=2, space="PSUM") as pyp:
                psum_vw = pvwp.tile([gpc, D], F32, space="PSUM")
                nc.tensor.matmul(psum_vw[:], vmeanT[:], w_t[:],
                                 start=True, stop=True)
                nc.vector.tensor_copy(vw16[:], psum_vw[:])

                for w in range(NW):
                    bt_t = p3.tile([gpc, P], F16, tag="bt")
                    nc.sync.dma_start(out=bt_t[:],
                                      in_=bt_d[:, w * P:(w + 1) * P])
                    psum_y = pyp.tile([P, D], F32, space="PSUM", tag="py")
                    nc.tensor.matmul(psum_y[:], out_T[:, w, :], w_t[:],
                                     start=True, stop=False,
                                     skip_group_check=True)
                    nc.tensor.matmul(psum_y[:], bt_t[:], vw16[:],
                                     start=False, stop=True,
                                     skip_group_check=True)
                    y_t = p3.tile([P, D], F32, tag="Y")
                    nc.scalar.activation(y_t[:], psum_y[:],
                                         mybir.ActivationFunctionType.Relu)
                    nc.sync.dma_start(out=y_d[w * P:(w + 1) * P, :],
                                      in_=y_t[:])
    _finish_compile(nc)
    return nc


def _finish_compile(nc):
    nc.compile()
    # compile()'s tail passes (library-load insertion for the custom DMA
    # instructions) can reintroduce >1 sync wait per instruction, which the
    # TRN2 ISA rejects. Re-split and re-codegen.
    import bass_rust
    bass_rust.generate_event_semaphores(nc)
    nc.codegen_inst_isa_subclasses()


_BUILD_CACHE = {}


def _build_cached(params):
    key = tuple(sorted((k, str(v)) for k, v in params.items()))
    if key not in _BUILD_CACHE:
        _BUILD_CACHE[key] = _build(params)
    return _BUILD_CACHE[key]


def _run(H, edge_index, batch, W, n_graphs, trace=False):
    H = np.asarray(H)
    params, in_maps, n_c, core_start = _prep(H, edge_index, batch, n_graphs)
    consts = _consts(params, np.asarray(W))
    for m in in_maps:
        m.update(consts)
    nc = _build_cached(params)
    res = run_bass_kernel_spmd(nc, in_maps, list(range(N_CORES)), trace=trace)
    N = H.shape[0]
    y = np.empty((N, D), dtype=np.float32)
    for c in range(N_CORES):
        y[core_start[c]:core_start[c] + n_c[c]] = \
            res.results[c]["y"][:n_c[c]]
    return y, res


def kernel(H, edge_index, batch, W):
    y, _ = _run(H, edge_index, batch, W, n_graphs=256,
                trace=bool(os.environ.get("GCN_TRACE")))
    return y


# revision 34
# speedup vs baseline: 1.4276x; 1.0049x over previous
"""GCN layer with virtual node on 8 Trainium2 NeuronCores (Bass/Tile).

Reference computation (fp32):
    agg = segment_sum(H[src], dst, N)        # message passing
    out = H + agg
    vmean = segment_mean(out, batch, G)      # virtual node
    out = out + vmean[batch]
    y = relu(out @ W)

Distribution strategy (self-contained, hardcoded):
  - batch is sorted, G=256 graphs, 8 cores -> core c owns graphs
    [32c, 32c+32) == a contiguous node range (graph-aligned node sharding).
    Per-graph means never cross cores: no collectives needed.
  - Edges partitioned by owning core of dst (host-side index arithmetic).
    Within a core: 128-dst windows. Source rows are fetched from a DRAM fp16
    copy of H with gpsimd dma_gather (int16 indices -> 4 source "classes" of
    <=32768 rows). Gather calls are grouped over GW windows per class and
    spread round-robin over 4 SWDGE queues so descriptor generation runs on
    all four Q7 core pairs concurrently (measured ~3.2x vs single queue).
  - segment_sum via PE one-hot matmul in TRANSPOSED orientation:
    psum_wT[f, dst] += G_t^T @ R_t, where R_t[s, m] = (drel[s,t] == m) is
    built on DVE (is_equal vs iota, batched over a whole gather call), so
    out_T[f, node] accumulates in SBUF and the final y = out^T... matmul
    needs no PE transpose.
  - virtual node: per window w, transpose out_T_w back (PE+ident) and
    accumulate psum_sT[f, g] += out_w^T @ b_w with host-baked one-hot
    b_w[node, graph]; vmeanT = psum_sT * (1/count); VW = vmeanT^T @ W.
  - final: psum_y = out_T_w^T @ W + Bt_w^T @ VW (host-baked Bt[g, node]),
    relu on ACT, DMA out.
"""
import os
import numpy as np

from concourse import bacc, mybir
import concourse.tile as tile
from concourse.bass_utils import run_bass_kernel_spmd

P = 128
N_CORES = 8
D = 128
F32 = mybir.dt.float32
I16 = mybir.dt.int16
F16 = mybir.dt.float16
CLASS_SIZE = 32768  # int16 index reach for dma_gather
GW = 3              # windows per gather group (keeps calls under the
                    # 16KB/lane SWDGE ring: ~2300 idxs -> ~145 descs/lane)
NQ = 4              # SWDGE queues


def _ceil(a, b):
    return -(-a // b)


# ---------------------------------------------------------------------------
# host-side prep: pure index arithmetic / sharding metadata
# ---------------------------------------------------------------------------

def _prep(H, edge_index, batch, n_graphs):
    N = H.shape[0]
    src = np.asarray(edge_index[0], dtype=np.int64)
    dst = np.asarray(edge_index[1], dtype=np.int64)
    batch = np.asarray(batch, dtype=np.int64)
    gpc = n_graphs // N_CORES
    n_cls = _ceil(N, CLASS_SIZE)

    gstart = np.searchsorted(batch, np.arange(n_graphs + 1))
    core_start = gstart[::gpc]  # [N_CORES+1]
    counts = np.diff(gstart)

    node_core = (batch // gpc).astype(np.int64)
    ecore = node_core[dst]

    n_c = np.diff(core_start)
    NW = int(_ceil(n_c.max(), P))

    # per-core (w, k) counts and sorted edge runs
    cnt = np.zeros((N_CORES, NW, n_cls), dtype=np.int64)
    runs = []
    for c in range(N_CORES):
        m = ecore == c
        s_c, d_c = src[m], dst[m]
        dstl = d_c - core_start[c]
        w = dstl >> 7
        k = s_c // CLASS_SIZE
        key = w * n_cls + k
        # src-sorted within each (w, k) run: ascending gather addresses
        # give the SDMA engines better HBM row/bank locality.
        order = np.lexsort((s_c, key))
        s_c, dstl, key = s_c[order], dstl[order], key[order]
        cnt[c] = np.bincount(key, minlength=NW * n_cls).reshape(NW, n_cls)
        runs.append((s_c, dstl, key))

    tiles_wk = _ceil(cnt.max(axis=0), P)            # [NW, n_cls]
    tiles_wk[:, 0] = np.maximum(tiles_wk[:, 0], 1)  # every window has >=1 tile

    groups = [(lo, min(lo + GW, NW)) for lo in range(0, NW, GW)]
    # layout: for g: for k (rotated per group): for w in group -> tiles.
    # The class order rotates so the round-robin SWDGE queue assignment
    # (queue = issue order % 4, which must stay in lockstep with Tile's
    # DMASW sem-lane rotation) sees balanced work per queue despite the
    # small last class.
    calls = []  # (gi, k, call_t0, nt, [(w, wt0, ntw), ...])
    wt0 = np.zeros((NW, n_cls), dtype=np.int64)
    t = 0
    for gi, (lo, hi) in enumerate(groups):
        for j in range(n_cls):
            k = (j + gi) % n_cls
            c_t0 = t
            wl = []
            for w in range(lo, hi):
                wt0[w, k] = t
                wl.append((w, t, int(tiles_wk[w, k])))
                t += int(tiles_wk[w, k])
            calls.append((gi, k, c_t0, t - c_t0, wl))
    T = int(t)
    RT = max(nt for _, _, _, nt, _ in calls)

    params = dict(
        N=N, NW=NW, T=T, RT=RT, gpc=gpc, n_cls=n_cls,
        tiles=tuple(tuple(int(x) for x in row) for row in tiles_wk),
        groups=tuple(groups),
        cls_size=tuple(min(CLASS_SIZE, N - CLASS_SIZE * k)
                       for k in range(n_cls)),
    )

    in_maps = []
    h16 = np.ascontiguousarray(H, dtype=np.float16)
    for c in range(N_CORES):
        s_c, dstl, key = runs[c]
        idx_flat = np.full(T * P, -1, dtype=np.int64)
        drel = np.full(T * P, -1.0, dtype=np.float32)
        # slot of each edge: base slot of its (w,k) run + offset within run
        cnt_c = cnt[c].ravel()
        run_start = np.concatenate([[0], np.cumsum(cnt_c)])
        off_in_run = np.arange(len(key)) - run_start[key]
        slot = wt0.ravel()[key] * P + off_in_run
        idx_flat[slot] = s_c - (key % n_cls) * CLASS_SIZE
        drel[slot] = (dstl & 127).astype(np.float32)
        # interior pads (before the last real edge of each call) -> idx 0.
        # num_idxs_reg must equal the per-call count of non-negative idxs
        # (the SWDGE ring reserves space from the register while Q7
        # generates from the trailing-trimmed idx list -- a mismatch
        # corrupts the ring bookkeeping and wedges the device).
        cnt32 = np.zeros(len(calls), dtype=np.int32)
        for ci, (_, _, c_t0, nt, _) in enumerate(calls):
            blk = idx_flat[c_t0 * P:(c_t0 + nt) * P]
            real = np.nonzero(blk >= 0)[0]
            if len(real):
                last = int(real[-1])
            else:
                blk[0] = 0  # keep every call non-empty
                last = 0
            pad = blk[:last + 1] < 0
            blk[:last + 1][pad] = 0
            cnt32[ci] = last + 1
        # wrap idx per call region into [16, nt*8] blocks
        wrapped = np.full((16, T * P // 16), -1, dtype=np.int16)
        for _, _, c_t0, nt, _ in calls:
            blk = idx_flat[c_t0 * P:(c_t0 + nt) * P]
            wrapped[:, c_t0 * 8:(c_t0 + nt) * 8] = \
                blk.reshape(nt * 8, 16).T.astype(np.int16)
        wrapped128 = np.ascontiguousarray(np.tile(wrapped, (8, 1)))

        drel16 = np.ascontiguousarray(
            drel.reshape(T, P).T.astype(np.float16))

        nodes = int(n_c[c])
        hcT = np.zeros((P, NW * P), dtype=np.float32)
        hcT[:, :nodes] = np.asarray(H[core_start[c]:core_start[c] + nodes],
                                    dtype=np.float32).T
        br = np.full(NW * P, -1, dtype=np.int64)
        br[:nodes] = batch[core_start[c]:core_start[c] + nodes] - c * gpc
        ball = (br.reshape(NW, P).T[:, :, None]
                == np.arange(gpc)[None, None, :]).astype(np.float16)
        bt = (np.arange(gpc)[:, None] == br[None, :]).astype(np.float16)
        invc = np.broadcast_to(
            (1.0 / np.maximum(counts[c * gpc:(c + 1) * gpc], 1)
             ).astype(np.float32)[None, :], (P, gpc)).copy()

        in_maps.append({
            "h16": h16,
            "cnt32": np.ascontiguousarray(cnt32.reshape(1, -1)),
            "idx16": wrapped128,
            "drel": drel16,
            "hct": np.ascontiguousarray(hcT),
            "ball": np.ascontiguousarray(ball.reshape(P, NW * gpc)),
            "bt": np.ascontiguousarray(bt),
            "invc": invc,
        })
    return params, in_maps, n_c, core_start


def _consts(params, W):
    RT = params["RT"]
    iota_rep = np.broadcast_to(np.arange(P, dtype=np.float16),
                               (P, RT, P)).reshape(P, RT * P).copy()
    ident = np.eye(P, dtype=np.float16)
    return {"iota_rep": iota_rep, "ident": ident,
            "w16": np.ascontiguousarray(W, dtype=np.float16)}


# ---------------------------------------------------------------------------
# device kernel builder (SPMD: one program, per-core data)
# ---------------------------------------------------------------------------

def _build(params):
    NW, T, RT = params["NW"], params["T"], params["RT"]
    gpc, n_cls = params["gpc"], params["n_cls"]
    tiles = params["tiles"]
    groups = params["groups"]
    cls_size = params["cls_size"]
    N = params["N"]

    # reconstruct the call layout (same order as _prep)
    calls = []
    wt0 = {}
    t = 0
    for gi, (lo, hi) in enumerate(groups):
        for j in range(n_cls):
            k = (j + gi) % n_cls
            c_t0 = t
            wl = []
            for w in range(lo, hi):
                wt0[(w, k)] = t
                wl.append((w, t, tiles[w][k]))
                t += tiles[w][k]
            calls.append((gi, k, c_t0, t - c_t0, wl))
    assert t == T

    # max tiles per group (for idx/drel staging buffers)
    gt_span = []
    for gi in range(len(groups)):
        g_t0 = calls[gi * n_cls][2]
        last = calls[gi * n_cls + n_cls - 1]
        gt_span.append((g_t0, last[2] + last[3]))
    GT = max(b - a for a, b in gt_span)
    RTk = [max(nt for _, k2, _, nt, _ in calls if k2 == k)
           for k in range(n_cls)]

    nc = bacc.Bacc("TRN2", target_bir_lowering=False, debug=False,
                   num_devices=N_CORES, num_swdge_queues=NQ)
    h16_d = nc.dram_tensor("h16", [N, D], F16, kind="ExternalInput")
    cnt_d = nc.dram_tensor("cnt32", [1, len(calls)], mybir.dt.int32,
                           kind="ExternalInput")
    idx_d = nc.dram_tensor("idx16", [P, T * P // 16], I16,
                           kind="ExternalInput")
    drel_d = nc.dram_tensor("drel", [P, T], F16, kind="ExternalInput")
    hct_d = nc.dram_tensor("hct", [P, NW * P], F32, kind="ExternalInput")
    ball_d = nc.dram_tensor("ball", [P, NW * gpc], F16, kind="ExternalInput")
    bt_d = nc.dram_tensor("bt", [gpc, NW * P], F16, kind="ExternalInput")
    invc_d = nc.dram_tensor("invc", [P, gpc], F32, kind="ExternalInput")
    iota_d = nc.dram_tensor("iota_rep", [P, RT * P], F16,
                            kind="ExternalInput")
    ident_d = nc.dram_tensor("ident", [P, P], F16, kind="ExternalInput")
    w_d = nc.dram_tensor("w16", [P, D], F16, kind="ExternalInput")
    y_d = nc.dram_tensor("y", [NW * P, D], F32, kind="ExternalOutput")

    with tile.TileContext(nc) as tc:
        with tc.tile_pool(name="const", bufs=1) as cpool:
            iota_t = cpool.tile([P, RT, P], F16)
            nc.sync.dma_start(out=iota_t[:], in_=iota_d[:])
            cnt_t = cpool.tile([1, len(calls)], mybir.dt.int32)
            nc.sync.dma_start(out=cnt_t[:], in_=cnt_d[:])
            ident_t = cpool.tile([P, P], F16)
            nc.sync.dma_start(out=ident_t[:], in_=ident_d[:])
            w_t = cpool.tile([P, D], F16)
            nc.sync.dma_start(out=w_t[:], in_=w_d[:])
            invc_t = cpool.tile([P, gpc], F32)
            nc.sync.dma_start(out=invc_t[:], in_=invc_d[:])

            out_T = cpool.tile([P, NW, P], F16)
            vmeanT = cpool.tile([P, gpc], F16)
            vw16 = cpool.tile([gpc, D], F16)

            from contextlib import ExitStack
            with ExitStack() as stack:
                ep = stack.enter_context
                idxp = ep(tc.tile_pool(name="idxp", bufs=3))
                drelp = ep(tc.tile_pool(name="drelp", bufs=3))
                gpools = [ep(tc.tile_pool(name=f"g{k}", bufs=3))
                          for k in range(n_cls)]
                rpools = [ep(tc.tile_pool(name=f"r{k}", bufs=3))
                          for k in range(n_cls)]
                hcp = ep(tc.tile_pool(name="hcp", bufs=6))
                bp = ep(tc.tile_pool(name="bp", bufs=3))
                onp = ep(tc.tile_pool(name="onp", bufs=3))
                pwp = ep(tc.tile_pool(name="pw", bufs=4, space="PSUM"))
                ptrp = ep(tc.tile_pool(name="ptr", bufs=2, space="PSUM"))
                psp = ep(tc.tile_pool(name="ps", bufs=1, space="PSUM"))
                gcnt = ep(nc.gpsimd.register("gcnt"))
                psum_sT = psp.tile([P, gpc], F32, space="PSUM")
                n_gather = 0

                for gi, (lo, hi) in enumerate(groups):
                    g_t0, g_t1 = gt_span[gi]
                    gcols = g_t1 - g_t0
                    idx_t = idxp.tile([P, GT * 8], I16, tag="idx")
                    nc.sync.dma_start(
                        out=idx_t[:, :gcols * 8],
                        in_=idx_d[:, g_t0 * 8:g_t1 * 8])
                    drel_t = drelp.tile([P, GT], F16, tag="drel")
                    nc.sync.dma_start(
                        out=drel_t[:, :gcols],
                        in_=drel_d[:, g_t0:g_t1])
                    # prefetch the group's H columns now, during the gather
                    # wait, instead of inside each window's critical chain
                    hcs = []
                    for w in range(lo, hi):
                        hc_t = hcp.tile([P, P], F32, tag="hc")
                        nc.sync.dma_start(out=hc_t[:],
                                          in_=hct_d[:, w * P:(w + 1) * P])
                        hcs.append(hc_t)

                    gts = {}
                    rts = {}
                    for j in range(n_cls):
                        _, k, c_t0, nt, _ = calls[gi * n_cls + j]
                        if nt == 0:
                            continue
                        g16 = gpools[k].tile([P, RTk[k], D], F16,
                                             tag=f"G{k}")
                        if gi < 3 or os.environ.get('GCN_SIM_MEMSET'):
                            # pad slots must be finite: 0 * NaN would poison
                            # the one-hot matmul. After the first rotation of
                            # the 3 buffers, stale content is old gathered
                            # rows (finite).
                            nc.vector.memset(g16[:], 0.0)
                        base = CLASS_SIZE * k
                        ci = gi * n_cls + j
                        nc.gpsimd.load(gcnt, cnt_t[0:1, ci:ci + 1])
                        nc.gpsimd.dma_gather(
                            out_ap=g16[:, :nt, :],
                            in_ap=h16_d[base:base + cls_size[k], :],
                            idxs_ap=idx_t[:, (c_t0 - g_t0) * 8:
                                          (c_t0 - g_t0 + nt) * 8],
                            num_idxs=nt * P,
                            num_idxs_reg=gcnt,
                            elem_size=D,
                            single_packet=False,
                            # queue stays in lockstep with Tile's DMASW
                            # sem-lane rotation (lane = issue order % 8,
                            # sems are locked to one SWDGE queue each)
                            queue_num=n_gather % NQ,
                        )
                        n_gather += 1
                        r16 = rpools[k].tile([P, RTk[k], P], F16,
                                             tag=f"R{k}")
                        nc.vector.tensor_tensor(
                            out=r16[:, :nt, :],
                            in0=drel_t[:, c_t0 - g_t0:c_t0 - g_t0 + nt
                                       ].to_broadcast([P, nt, P]),
                            in1=iota_t[:, :nt, :],
                            op=mybir.AluOpType.is_equal)
                        gts[k] = (g16, c_t0)
                        rts[k] = r16

                    for w in range(lo, hi):
                        psum_w = pwp.tile([P, P], F32, space="PSUM",
                                          tag="pw")
                        seq = []
                        for k in range(n_cls):
                            _, c_t0 = gts[k][0], gts[k][1]
                            for j in range(tiles[w][k]):
                                seq.append((k, wt0[(w, k)] - c_t0 + j))
                        for si, (k, j) in enumerate(seq):
                            nc.tensor.matmul(
                                psum_w[:], gts[k][0][:, j, :],
                                rts[k][:, j, :],
                                start=(si == 0), stop=(si == len(seq) - 1))
                        nc.vector.tensor_tensor(
                            out=out_T[:, w, :], in0=psum_w[:],
                            in1=hcs[w - lo][:],
                            op=mybir.AluOpType.add)
                        # transpose back for the virtual-node segment sum
                        ptr_t = ptrp.tile([P, P], F16, space="PSUM",
                                          tag="ptr")
                        nc.tensor.transpose(ptr_t[:], out_T[:, w, :],
                                            ident_t[:])
                        on16 = onp.tile([P, P], F16, tag="on")
                        nc.vector.tensor_copy(on16[:], ptr_t[:])
                        b_t = bp.tile([P, gpc], F16, tag="b")
                        nc.sync.dma_start(
                            out=b_t[:],
                            in_=ball_d[:, w * gpc:(w + 1) * gpc])
                        nc.tensor.matmul(
                            psum_sT[:], on16[:], b_t[:],
                            start=(w == 0), stop=(w == NW - 1),
                            skip_group_check=True)

                nc.vector.tensor_tensor(
                    out=vmeanT[:], in0=psum_sT[:], in1=invc_t[:],
                    op=mybir.AluOpType.mult)

            with tc.tile_pool(name="p3", bufs=3) as p3, \
                 tc.tile_pool(name="pvw", bufs=1, space="PSUM") as pvwp, \
                 tc.tile_pool(name="py", bufs=2, space="PSUM") as pyp:
                psum_vw = pvwp.tile([gpc, D], F32, space="PSUM")
                nc.tensor.matmul(psum_vw[:], vmeanT[:], w_t[:],
                                 start=True, stop=True)
                nc.vector.tensor_copy(vw16[:], psum_vw[:])

                for w in range(NW):
                    bt_t = p3.tile([gpc, P], F16, tag="bt")
                    nc.sync.dma_start(out=bt_t[:],
                                      in_=bt_d[:, w * P:(w + 1) * P])
                    psum_y = pyp.tile([P, D], F32, space="PSUM", tag="py")
                    nc.tensor.matmul(psum_y[:], out_T[:, w, :], w_t[:],
                                     start=True, stop=False,
                                     skip_group_check=True)
                    nc.tensor.matmul(psum_y[:], bt_t[:], vw16[:],
                                     start=False, stop=True,
                                     skip_group_check=True)
                    y_t = p3.tile([P, D], F32, tag="Y")
                    nc.scalar.activation(y_t[:], psum_y[:],
                                         mybir.ActivationFunctionType.Relu)
                    nc.sync.dma_start(out=y_d[w * P:(w + 1) * P, :],
                                      in_=y_t[:])
    _finish_compile(nc)
    return nc


def _finish_compile(nc):
    nc.compile()
    # compile()'s tail passes (library-load insertion for the custom DMA
    # instructions) can reintroduce >1 sync wait per instruction, which the
    # TRN2 ISA rejects. Re-split and re-codegen.
    import bass_rust
    bass_rust.generate_event_semaphores(nc)
    nc.codegen_inst_isa_subclasses()


_BUILD_CACHE = {}


def _build_cached(params):
    key = tuple(sorted((k, str(v)) for k, v in params.items()))
    if key not in _BUILD_CACHE:
        _BUILD_CACHE[key] = _build(params)
    return _BUILD_CACHE[key]


def _run(H, edge_index, batch, W, n_graphs, trace=False):
    H = np.asarray(H)
    params, in_maps, n_c, core_start = _prep(H, edge_index, batch, n_graphs)
    consts = _consts(params, np.asarray(W))
    for m in in_maps:
        m.update(consts)
    nc = _build_cached(params)
    res = run_bass_kernel_spmd(nc, in_maps, list(range(N_CORES)), trace=trace)
    N = H.shape[0]
    y = np.empty((N, D), dtype=np.float32)
    for c in range(N_CORES):
        y[core_start[c]:core_start[c] + n_c[c]] = \
            res.results[c]["y"][:n_c[c]]
    return y, res


def kernel(H, edge_index, batch, W):
    y, _ = _run(H, edge_index, batch, W, n_graphs=256,
                trace=bool(os.environ.get("GCN_TRACE")))
    return y


# revision 36
# speedup vs baseline: 1.5930x; 1.1159x over previous
"""GCN layer with virtual node on 8 Trainium2 NeuronCores (Bass/Tile).

Reference computation (fp32):
    agg = segment_sum(H[src], dst, N)        # message passing
    out = H + agg
    vmean = segment_mean(out, batch, G)      # virtual node
    out = out + vmean[batch]
    y = relu(out @ W)

Distribution strategy (self-contained, hardcoded):
  - batch is sorted, G=256 graphs, 8 cores -> core c owns graphs
    [32c, 32c+32) == a contiguous node range (graph-aligned node sharding).
    Per-graph means never cross cores: no collectives needed.
  - Edges partitioned by owning core of dst (host-side index arithmetic).
    Within a core: 128-dst windows. Source rows are fetched from a DRAM fp16
    copy of H with gpsimd dma_gather (int16 indices -> 4 source "classes" of
    <=32768 rows). Gather calls are grouped over GW windows per class and
    spread round-robin over 4 SWDGE queues so descriptor generation runs on
    all four Q7 core pairs concurrently (measured ~3.2x vs single queue).
  - segment_sum via PE one-hot matmul in TRANSPOSED orientation:
    psum_wT[f, dst] += G_t^T @ R_t, where R_t[s, m] = (drel[s,t] == m) is
    built on DVE (is_equal vs iota, batched over a whole gather call), so
    out_T[f, node] accumulates in SBUF and the final y = out^T... matmul
    needs no PE transpose.
  - virtual node: per window w, transpose out_T_w back (PE+ident) and
    accumulate psum_sT[f, g] += out_w^T @ b_w with host-baked one-hot
    b_w[node, graph]; vmeanT = psum_sT * (1/count); VW = vmeanT^T @ W.
  - final: psum_y = out_T_w^T @ W + Bt_w^T @ VW (host-baked Bt[g, node]),
    relu on ACT, DMA out.
"""
import os
import numpy as np

from concourse import bacc, mybir
import concourse.tile as tile
from concourse.bass_utils import run_bass_kernel_spmd

P = 128
N_CORES = 8
D = 128
F32 = mybir.dt.float32
I16 = mybir.dt.int16
F16 = mybir.dt.float16
CLASS_SIZE = 32768  # int16 index reach for dma_gather
GW = 2              # windows per gather group (keeps calls under the
                    # 16KB/lane SWDGE ring; small groups shrink the
                    # group-sync quantum of the Q7-bound gather phase)
NQ = 4              # SWDGE queues


def _ceil(a, b):
    return -(-a // b)


# ---------------------------------------------------------------------------
# host-side prep: pure index arithmetic / sharding metadata
# ---------------------------------------------------------------------------

def _prep(H, edge_index, batch, n_graphs):
    N = H.shape[0]
    src = np.asarray(edge_index[0], dtype=np.int64)
    dst = np.asarray(edge_index[1], dtype=np.int64)
    batch = np.asarray(batch, dtype=np.int64)
    gpc = n_graphs // N_CORES
    n_cls = _ceil(N, CLASS_SIZE)

    gstart = np.searchsorted(batch, np.arange(n_graphs + 1))
    core_start = gstart[::gpc]  # [N_CORES+1]
    counts = np.diff(gstart)

    node_core = (batch // gpc).astype(np.int64)
    ecore = node_core[dst]

    n_c = np.diff(core_start)
    NW = int(_ceil(n_c.max(), P))

    # per-core (w, k) counts and sorted edge runs
    cnt = np.zeros((N_CORES, NW, n_cls), dtype=np.int64)
    runs = []
    for c in range(N_CORES):
        m = ecore == c
        s_c, d_c = src[m], dst[m]
        dstl = d_c - core_start[c]
        w = dstl >> 7
        k = s_c // CLASS_SIZE
        key = w * n_cls + k
        # src-sorted within each (w, k) run: ascending gather addresses
        # give the SDMA engines better HBM row/bank locality.
        order = np.lexsort((s_c, key))
        s_c, dstl, key = s_c[order], dstl[order], key[order]
        cnt[c] = np.bincount(key, minlength=NW * n_cls).reshape(NW, n_cls)
        runs.append((s_c, dstl, key))

    tiles_wk = _ceil(cnt.max(axis=0), P)            # [NW, n_cls]
    tiles_wk[:, 0] = np.maximum(tiles_wk[:, 0], 1)  # every window has >=1 tile

    groups = [(lo, min(lo + GW, NW)) for lo in range(0, NW, GW)]
    # layout: for g: for k (rotated per group): for w in group -> tiles.
    # The class order rotates so the round-robin SWDGE queue assignment
    # (queue = issue order % 4, which must stay in lockstep with Tile's
    # DMASW sem-lane rotation) sees balanced work per queue despite the
    # small last class.
    calls = []  # (gi, k, call_t0, nt, [(w, wt0, ntw), ...])
    wt0 = np.zeros((NW, n_cls), dtype=np.int64)
    t = 0
    for gi, (lo, hi) in enumerate(groups):
        for j in range(n_cls):
            k = (j + gi) % n_cls
            c_t0 = t
            wl = []
            for w in range(lo, hi):
                wt0[w, k] = t
                wl.append((w, t, int(tiles_wk[w, k])))
                t += int(tiles_wk[w, k])
            calls.append((gi, k, c_t0, t - c_t0, wl))
    T = int(t)
    RT = max(nt for _, _, _, nt, _ in calls)

    params = dict(
        N=N, NW=NW, T=T, RT=RT, gpc=gpc, n_cls=n_cls,
        tiles=tuple(tuple(int(x) for x in row) for row in tiles_wk),
        groups=tuple(groups),
        cls_size=tuple(min(CLASS_SIZE, N - CLASS_SIZE * k)
                       for k in range(n_cls)),
    )

    in_maps = []
    h16 = np.ascontiguousarray(H, dtype=np.float16)
    for c in range(N_CORES):
        s_c, dstl, key = runs[c]
        idx_flat = np.full(T * P, -1, dtype=np.int64)
        drel = np.full(T * P, -1.0, dtype=np.float32)
        # slot of each edge: base slot of its (w,k) run + offset within run
        cnt_c = cnt[c].ravel()
        run_start = np.concatenate([[0], np.cumsum(cnt_c)])
        off_in_run = np.arange(len(key)) - run_start[key]
        slot = wt0.ravel()[key] * P + off_in_run
        idx_flat[slot] = s_c - (key % n_cls) * CLASS_SIZE
        drel[slot] = (dstl & 127).astype(np.float32)
        # interior pads (before the last real edge of each call) -> idx 0.
        # num_idxs_reg must equal the per-call count of non-negative idxs
        # (the SWDGE ring reserves space from the register while Q7
        # generates from the trailing-trimmed idx list -- a mismatch
        # corrupts the ring bookkeeping and wedges the device).
        cnt32 = np.zeros(len(calls), dtype=np.int32)
        for ci, (_, _, c_t0, nt, _) in enumerate(calls):
            blk = idx_flat[c_t0 * P:(c_t0 + nt) * P]
            real = np.nonzero(blk >= 0)[0]
            if len(real):
                last = int(real[-1])
            else:
                blk[0] = 0  # keep every call non-empty
                last = 0
            pad = blk[:last + 1] < 0
            blk[:last + 1][pad] = 0
            cnt32[ci] = last + 1
        # wrap idx per call region into [16, nt*8] blocks
        wrapped = np.full((16, T * P // 16), -1, dtype=np.int16)
        for _, _, c_t0, nt, _ in calls:
            blk = idx_flat[c_t0 * P:(c_t0 + nt) * P]
            wrapped[:, c_t0 * 8:(c_t0 + nt) * 8] = \
                blk.reshape(nt * 8, 16).T.astype(np.int16)
        wrapped128 = np.ascontiguousarray(np.tile(wrapped, (8, 1)))

        drel16 = np.ascontiguousarray(
            drel.reshape(T, P).T.astype(np.float16))

        nodes = int(n_c[c])
        hcT = np.zeros((P, NW * P), dtype=np.float32)
        hcT[:, :nodes] = np.asarray(H[core_start[c]:core_start[c] + nodes],
                                    dtype=np.float32).T
        br = np.full(NW * P, -1, dtype=np.int64)
        br[:nodes] = batch[core_start[c]:core_start[c] + nodes] - c * gpc
        ball = (br.reshape(NW, P).T[:, :, None]
                == np.arange(gpc)[None, None, :]).astype(np.float16)
        bt = (np.arange(gpc)[:, None] == br[None, :]).astype(np.float16)
        invc = np.broadcast_to(
            (1.0 / np.maximum(counts[c * gpc:(c + 1) * gpc], 1)
             ).astype(np.float32)[None, :], (P, gpc)).copy()

        in_maps.append({
            "h16": h16,
            "cnt32": np.ascontiguousarray(cnt32.reshape(1, -1)),
            "idx16": wrapped128,
            "drel": drel16,
            "hct": np.ascontiguousarray(hcT),
            "ball": np.ascontiguousarray(ball.reshape(P, NW * gpc)),
            "bt": np.ascontiguousarray(bt),
            "invc": invc,
        })
    return params, in_maps, n_c, core_start


def _consts(params, W):
    RT = params["RT"]
    iota_rep = np.broadcast_to(np.arange(P, dtype=np.float16),
                               (P, RT, P)).reshape(P, RT * P).copy()
    ident = np.eye(P, dtype=np.float16)
    return {"iota_rep": iota_rep, "ident": ident,
            "w16": np.ascontiguousarray(W, dtype=np.float16)}


# ---------------------------------------------------------------------------
# device kernel builder (SPMD: one program, per-core data)
# ---------------------------------------------------------------------------

def _build(params):
    NW, T, RT = params["NW"], params["T"], params["RT"]
    gpc, n_cls = params["gpc"], params["n_cls"]
    tiles = params["tiles"]
    groups = params["groups"]
    cls_size = params["cls_size"]
    N = params["N"]

    # reconstruct the call layout (same order as _prep)
    calls = []
    wt0 = {}
    t = 0
    for gi, (lo, hi) in enumerate(groups):
        for j in range(n_cls):
            k = (j + gi) % n_cls
            c_t0 = t
            wl = []
            for w in range(lo, hi):
                wt0[(w, k)] = t
                wl.append((w, t, tiles[w][k]))
                t += tiles[w][k]
            calls.append((gi, k, c_t0, t - c_t0, wl))
    assert t == T

    # max tiles per group (for idx/drel staging buffers)
    gt_span = []
    for gi in range(len(groups)):
        g_t0 = calls[gi * n_cls][2]
        last = calls[gi * n_cls + n_cls - 1]
        gt_span.append((g_t0, last[2] + last[3]))
    GT = max(b - a for a, b in gt_span)
    RTk = [max(nt for _, k2, _, nt, _ in calls if k2 == k)
           for k in range(n_cls)]

    nc = bacc.Bacc("TRN2", target_bir_lowering=False, debug=False,
                   num_devices=N_CORES, num_swdge_queues=NQ)
    h16_d = nc.dram_tensor("h16", [N, D], F16, kind="ExternalInput")
    cnt_d = nc.dram_tensor("cnt32", [1, len(calls)], mybir.dt.int32,
                           kind="ExternalInput")
    idx_d = nc.dram_tensor("idx16", [P, T * P // 16], I16,
                           kind="ExternalInput")
    drel_d = nc.dram_tensor("drel", [P, T], F16, kind="ExternalInput")
    hct_d = nc.dram_tensor("hct", [P, NW * P], F32, kind="ExternalInput")
    ball_d = nc.dram_tensor("ball", [P, NW * gpc], F16, kind="ExternalInput")
    bt_d = nc.dram_tensor("bt", [gpc, NW * P], F16, kind="ExternalInput")
    invc_d = nc.dram_tensor("invc", [P, gpc], F32, kind="ExternalInput")
    iota_d = nc.dram_tensor("iota_rep", [P, RT * P], F16,
                            kind="ExternalInput")
    ident_d = nc.dram_tensor("ident", [P, P], F16, kind="ExternalInput")
    w_d = nc.dram_tensor("w16", [P, D], F16, kind="ExternalInput")
    y_d = nc.dram_tensor("y", [NW * P, D], F32, kind="ExternalOutput")

    with tile.TileContext(nc) as tc:
        with tc.tile_pool(name="const", bufs=1) as cpool:
            iota_t = cpool.tile([P, RT, P], F16)
            nc.sync.dma_start(out=iota_t[:], in_=iota_d[:])
            cnt_t = cpool.tile([1, len(calls)], mybir.dt.int32)
            nc.sync.dma_start(out=cnt_t[:], in_=cnt_d[:])
            ident_t = cpool.tile([P, P], F16)
            nc.sync.dma_start(out=ident_t[:], in_=ident_d[:])
            w_t = cpool.tile([P, D], F16)
            nc.sync.dma_start(out=w_t[:], in_=w_d[:])
            invc_t = cpool.tile([P, gpc], F32)
            nc.sync.dma_start(out=invc_t[:], in_=invc_d[:])

            out_T = cpool.tile([P, NW, P], F16)
            vmeanT = cpool.tile([P, gpc], F16)
            vw16 = cpool.tile([gpc, D], F16)

            from contextlib import ExitStack
            with ExitStack() as stack:
                ep = stack.enter_context
                idxp = ep(tc.tile_pool(name="idxp", bufs=3))
                drelp = ep(tc.tile_pool(name="drelp", bufs=3))
                gpools = [ep(tc.tile_pool(name=f"g{k}", bufs=3))
                          for k in range(n_cls)]
                rpools = [ep(tc.tile_pool(name=f"r{k}", bufs=3))
                          for k in range(n_cls)]
                hcp = ep(tc.tile_pool(name="hcp", bufs=3))
                bp = ep(tc.tile_pool(name="bp", bufs=3))
                onp = ep(tc.tile_pool(name="onp", bufs=3))
                pwp = ep(tc.tile_pool(name="pw", bufs=4, space="PSUM"))
                ptrp = ep(tc.tile_pool(name="ptr", bufs=2, space="PSUM"))
                psp = ep(tc.tile_pool(name="ps", bufs=1, space="PSUM"))
                gcnt = ep(nc.gpsimd.register("gcnt"))
                psum_sT = psp.tile([P, gpc], F32, space="PSUM")
                n_gather = 0

                for gi, (lo, hi) in enumerate(groups):
                    g_t0, g_t1 = gt_span[gi]
                    gcols = g_t1 - g_t0
                    idx_t = idxp.tile([P, GT * 8], I16, tag="idx")
                    nc.sync.dma_start(
                        out=idx_t[:, :gcols * 8],
                        in_=idx_d[:, g_t0 * 8:g_t1 * 8])
                    drel_t = drelp.tile([P, GT], F16, tag="drel")
                    nc.sync.dma_start(
                        out=drel_t[:, :gcols],
                        in_=drel_d[:, g_t0:g_t1])

                    gts = {}
                    rts = {}
                    for j in range(n_cls):
                        _, k, c_t0, nt, _ = calls[gi * n_cls + j]
                        if nt == 0:
                            continue
                        g16 = gpools[k].tile([P, RTk[k], D], F16,
                                             tag=f"G{k}")
                        if gi < 3 or os.environ.get('GCN_SIM_MEMSET'):
                            # pad slots must be finite: 0 * NaN would poison
                            # the one-hot matmul. After the first rotation of
                            # the 3 buffers, stale content is old gathered
                            # rows (finite).
                            nc.vector.memset(g16[:], 0.0)
                        base = CLASS_SIZE * k
                        ci = gi * n_cls + j
                        nc.gpsimd.load(gcnt, cnt_t[0:1, ci:ci + 1])
                        nc.gpsimd.dma_gather(
                            out_ap=g16[:, :nt, :],
                            in_ap=h16_d[base:base + cls_size[k], :],
                            idxs_ap=idx_t[:, (c_t0 - g_t0) * 8:
                                          (c_t0 - g_t0 + nt) * 8],
                            num_idxs=nt * P,
                            num_idxs_reg=gcnt,
                            elem_size=D,
                            single_packet=False,
                            # queue stays in lockstep with Tile's DMASW
                            # sem-lane rotation (lane = issue order % 8,
                            # sems are locked to one SWDGE queue each)
                            queue_num=n_gather % NQ,
                        )
                        n_gather += 1
                        r16 = rpools[k].tile([P, RTk[k], P], F16,
                                             tag=f"R{k}")
                        nc.vector.tensor_tensor(
                            out=r16[:, :nt, :],
                            in0=drel_t[:, c_t0 - g_t0:c_t0 - g_t0 + nt
                                       ].to_broadcast([P, nt, P]),
                            in1=iota_t[:, :nt, :],
                            op=mybir.AluOpType.is_equal)
                        gts[k] = (g16, c_t0)
                        rts[k] = r16

                    for w in range(lo, hi):
                        psum_w = pwp.tile([P, P], F32, space="PSUM",
                                          tag="pw")
                        seq = []
                        for k in range(n_cls):
                            _, c_t0 = gts[k][0], gts[k][1]
                            for j in range(tiles[w][k]):
                                seq.append((k, wt0[(w, k)] - c_t0 + j))
                        for si, (k, j) in enumerate(seq):
                            nc.tensor.matmul(
                                psum_w[:], gts[k][0][:, j, :],
                                rts[k][:, j, :],
                                start=(si == 0), stop=(si == len(seq) - 1))
                        hc_t = hcp.tile([P, P], F32, tag="hc")
                        nc.sync.dma_start(out=hc_t[:],
                                          in_=hct_d[:, w * P:(w + 1) * P])
                        nc.vector.tensor_tensor(
                            out=out_T[:, w, :], in0=psum_w[:], in1=hc_t[:],
                            op=mybir.AluOpType.add)
                        # transpose back for the virtual-node segment sum
                        ptr_t = ptrp.tile([P, P], F16, space="PSUM",
                                          tag="ptr")
                        nc.tensor.transpose(ptr_t[:], out_T[:, w, :],
                                            ident_t[:])
                        on16 = onp.tile([P, P], F16, tag="on")
                        nc.vector.tensor_copy(on16[:], ptr_t[:])
                        b_t = bp.tile([P, gpc], F16, tag="b")
                        nc.sync.dma_start(
                            out=b_t[:],
                            in_=ball_d[:, w * gpc:(w + 1) * gpc])
                        nc.tensor.matmul(
                            psum_sT[:], on16[:], b_t[:],
                            start=(w == 0), stop=(w == NW - 1),
                            skip_group_check=True)

                nc.vector.tensor_tensor(
                    out=vmeanT[:], in0=psum_sT[:], in1=invc_t[:],
                    op=mybir.AluOpType.mult)

            with tc.tile_pool(name="p3", bufs=3) as p3, \
                 tc.tile_pool(name="pvw", bufs=1, space="PSUM") as pvwp, \
                 tc.tile_pool(name="py", bufs=2, space="PSUM") as pyp:
                psum_vw = pvwp.tile([gpc, D], F32, space="PSUM")
                nc.tensor.matmul(psum_vw[:], vmeanT[:], w_t[:],
                                 start=True, stop=True)
                nc.vector.tensor_copy(vw16[:], psum_vw[:])

                for w in range(NW):
                    bt_t = p3.tile([gpc, P], F16, tag="bt")
                    nc.sync.dma_start(out=bt_t[:],
                                      in_=bt_d[:, w * P:(w + 1) * P])
                    psum_y = pyp.tile([P, D], F32, space="PSUM", tag="py")
                    nc.tensor.matmul(psum_y[:], out_T[:, w, :], w_t[:],
                                     start=True, stop=False,
                                     skip_group_check=True)
                    nc.tensor.matmul(psum_y[:], bt_t[:], vw16[:],
                                     start=False, stop=True,
                                     skip_group_check=True)
                    y_t = p3.tile([P, D], F32, tag="Y")
                    nc.scalar.activation(y_t[:], psum_y[:],
                                         mybir.ActivationFunctionType.Relu)
                    nc.sync.dma_start(out=y_d[w * P:(w + 1) * P, :],
                                      in_=y_t[:])
    _finish_compile(nc)
    return nc


def _finish_compile(nc):
    nc.compile()
    # compile()'s tail passes (library-load insertion for the custom DMA
    # instructions) can reintroduce >1 sync wait per instruction, which the
    # TRN2 ISA rejects. Re-split and re-codegen.
    import bass_rust
    bass_rust.generate_event_semaphores(nc)
    nc.codegen_inst_isa_subclasses()


_BUILD_CACHE = {}


def _build_cached(params):
    key = tuple(sorted((k, str(v)) for k, v in params.items()))
    if key not in _BUILD_CACHE:
        _BUILD_CACHE[key] = _build(params)
    return _BUILD_CACHE[key]


def _run(H, edge_index, batch, W, n_graphs, trace=False):
    H = np.asarray(H)
    params, in_maps, n_c, core_start = _prep(H, edge_index, batch, n_graphs)
    consts = _consts(params, np.asarray(W))
    for m in in_maps:
        m.update(consts)
    nc = _build_cached(params)
    res = run_bass_kernel_spmd(nc, in_maps, list(range(N_CORES)), trace=trace)
    N = H.shape[0]
    y = np.empty((N, D), dtype=np.float32)
    for c in range(N_CORES):
        y[core_start[c]:core_start[c] + n_c[c]] = \
            res.results[c]["y"][:n_c[c]]
    return y, res


def kernel(H, edge_index, batch, W):
    y, _ = _run(H, edge_index, batch, W, n_graphs=256,
                trace=bool(os.environ.get("GCN_TRACE")))
    return y
